# revision 37
# baseline (speedup 1.0000x reference)
"""Trainium2 Bass kernel for nn_CRNN: 3 stacked ConvGRU cells (applied once,
zero initial hidden state) + dense head, B=65536 samples of [1,3,3].

Math: with h=0 the GRU cell reduces to
    x_out = tanh(conv3(x, Wo[:, :cin]) + bo) * sigmoid(conv3(x, Wu[:, :cin]) + bu)
(the reset gate is dead).  A 3x3 SAME conv on a 3x3 image is a dense linear map
on the flattened [cin*9] feature vector, so the whole net is a 4-layer MLP over
features 9 -> 288 -> 576 -> 144 -> 9 with gate products between layers.

Kernel layout: features on partitions, batch on the free dim.  Pure data
parallel over 8 cores (8192 samples each).  Matmuls run in fp16 (FP22
multiply, fp32 PSUM accumulate), activations on ScalarE straight out of PSUM,
gate product on VectorE (fp32 result stored, fp16 copy feeds the next layer).

Outputs are stored [features, batch] contiguous; the host transposes on
unshard (that IS the unshard gather).
"""

import os
import sys

import numpy as np

sys.path.insert(0, "/opt/trn_rl_repo")

import concourse.bass as bass
import concourse.mybir as mybir
import concourse.tile as tile
from concourse import bacc
from concourse.bass_utils import run_bass_kernel_spmd

# ---------------------------------------------------------------- config
N_CORES = 8
B_TOTAL = 65536
B_CORE = B_TOTAL // N_CORES
BT = 2048          # big batch tile (free-dim) per pipeline step
SUB = 512          # matmul free-dim (= one PSUM bank of fp32)
MM_DT = mybir.dt.float16    # matmul operand dtype
MM_NP = np.float16
STACK_TAILS = bool(int(os.environ.get("KERNEL_STACK_TAILS", "1")))
SPLIT_TAILS = bool(int(os.environ.get("KERNEL_SPLIT_TAILS", "0")))

F32 = mybir.dt.float32
AF = mybir.ActivationFunctionType

# layer geometry: K input feats, F gate width, nf = F//128 full chunks,
# r = F%128 tail rows, stride = 32-aligned stack stride, s = stacks.
LAYERS = [
    dict(K=9, F=288, nf=2, r=32, stride=32, s=4),
    dict(K=288, F=576, nf=4, r=64, stride=64, s=2),
    dict(K=576, F=144, nf=1, r=16, stride=32, s=4),
]
MERGED_TAILS = bool(int(os.environ.get("KERNEL_MERGED_TAILS", "0")))
# merged-tail layout per layer: blocks of [u(r); o(r)] stacked s2-fold at
# stride2 partitions, covering cps = BT // s2 batch columns per stack.
TAILG = [
    dict(r=32, stride=64, s=2, oo=32),    # L1: [u32|o32]       -> [128, 1024]
    dict(r=64, stride=128, s=1, oo=64),   # L2: [u64|o64]       -> [128, 2048]
    dict(r=16, stride=64, s=2, oo=32),    # L3: [u16|pad16|o16] -> [128, 1024]
]
DENSE_K, DENSE_F = 144, 9
CH = [32, 64, 16]
CIN = [1, 32, 64]


# ---------------------------------------------------------------- host-side prep
def _conv_to_dense(w, cin_used):
    """w: [Cout, Cin, 3, 3] OIHW SAME conv on 3x3 images -> A: [Cout*9, cin_used*9]
    with y_flat = A @ x_flat, flat feature index = c*9 + i*3 + j."""
    w = np.asarray(w, np.float32)[:, :cin_used]
    cout, cin = w.shape[0], w.shape[1]
    A = np.zeros((cout, 9, cin, 9), np.float32)
    for i in range(3):
        for j in range(3):
            for di in range(3):
                for dj in range(3):
                    si, sj = i + di - 1, j + dj - 1
                    if 0 <= si < 3 and 0 <= sj < 3:
                        A[:, i * 3 + j, :, si * 3 + sj] = w[:, :, di, dj]
    return A.reshape(cout * 9, cin * 9)


def _bias_pack(b_chan, L):
    """Per-channel bias -> [128, nf+1] fp32: col m = features m*128..m*128+127,
    last col = tail features replicated at each stack's partition offset."""
    nf, r, stride, s = L["nf"], L["r"], L["stride"], L["s"]
    bf = np.repeat(np.asarray(b_chan, np.float32), 9)
    out = np.zeros((128, nf + 1), np.float32)
    for m in range(nf):
        out[:, m] = bf[m * 128:(m + 1) * 128]
    for k in range(s if STACK_TAILS else 1):
        out[k * stride:k * stride + r, nf] = bf[L["F"] - r:]
    return out


def _prep_consts(inputs):
    c = {}
    for li, L in enumerate(LAYERS, start=1):
        for g in ("u", "o"):
            A = _conv_to_dense(inputs[f"w{li}{g}"], CIN[li - 1])      # [F, K]
            c[f"w{li}{g}"] = np.ascontiguousarray(A.T).astype(MM_NP)  # [K, F]
            c[f"b{li}{g}"] = _bias_pack(inputs[f"b{li}{g}"], LAYERS[li - 1])
    c["wd"] = np.ascontiguousarray(
        np.asarray(inputs["wd"], np.float32).T).astype(MM_NP)          # [144, 9]
    bd = np.zeros((128, 1), np.float32)
    for k in range(4 if STACK_TAILS else 1):
        bd[k * 32:k * 32 + DENSE_F, 0] = np.asarray(inputs["bd"], np.float32)
    c["bd"] = bd
    if MERGED_TAILS:
        for li, (L, T) in enumerate(zip(LAYERS, TAILG), start=1):
            r, st2, s2 = T["r"], T["stride"], T["s"]
            bu = np.repeat(np.asarray(inputs[f"b{li}u"], np.float32), 9)[L["F"] - r:]
            bo = np.repeat(np.asarray(inputs[f"b{li}o"], np.float32), 9)[L["F"] - r:]
            oo = T["oo"]
            bm = np.zeros((128, 1), np.float32)
            for k in range(s2):
                bm[k * st2:k * st2 + r, 0] = bu
                bm[k * st2 + oo:k * st2 + oo + r, 0] = bo
            c[f"bm{li}"] = bm
            sel = np.zeros((128, r), MM_NP)
            for k in range(s2):
                for i in range(r):
                    sel[k * st2 + oo + i, i] = 1.0
            c[f"sel{li}"] = sel
    return c


def _full_chunks(K):
    """[(lo, hi), ...] covering the full-128 part of K."""
    return [(m * 128, (m + 1) * 128) for m in range(K // 128)]


# ---------------------------------------------------------------- bass kernel
def build_bass(b_core=B_CORE):
    nc = bacc.Bacc("TRN2", target_bir_lowering=False, debug=False)
    nbt = b_core // BT
    assert b_core % BT == 0

    # ---- DRAM tensors
    x0_d = nc.dram_tensor("x0", [9, b_core], MM_DT, kind="ExternalInput").ap()
    wd_d, bd_d = {}, {}
    for li, L in enumerate(LAYERS, start=1):
        for g in ("u", "o"):
            wd_d[f"{li}{g}"] = nc.dram_tensor(
                f"w{li}{g}", [L["K"], L["F"]], MM_DT, kind="ExternalInput").ap()
            bd_d[f"{li}{g}"] = nc.dram_tensor(
                f"b{li}{g}", [128, L["nf"] + 1], F32, kind="ExternalInput").ap()
    wdd = nc.dram_tensor("wd", [DENSE_K, DENSE_F], MM_DT, kind="ExternalInput").ap()
    bdd = nc.dram_tensor("bd", [128, 1], F32, kind="ExternalInput").ap()
    bm_d = {}
    if MERGED_TAILS:
        for li in (1, 2, 3):
            bm_d[li] = nc.dram_tensor(
                f"bm{li}", [128, 1], F32, kind="ExternalInput").ap()
            bm_d[(li, "sel")] = nc.dram_tensor(
                f"sel{li}", [128, TAILG[li - 1]["r"]], MM_DT,
                kind="ExternalInput").ap()
    x_out = [
        nc.dram_tensor(f"x{li}", [L["F"], b_core], F32, kind="ExternalOutput").ap()
        for li, L in enumerate(LAYERS, start=1)
    ]
    y_d = nc.dram_tensor("y", [DENSE_F, b_core], F32, kind="ExternalOutput").ap()

    # geometry of each matmul-input source, in order: L1 in, L2 in, L3 in, dense in
    tg = TAILG if MERGED_TAILS else [
        dict(r=L["r"], stride=L["stride"], s=L["s"]) for L in LAYERS]
    in_geo = [dict(full_rows=[9], r=0, stride=0, s=0)] + [
        dict(full_rows=[128] * L["nf"], r=T["r"], stride=T["stride"], s=T["s"])
        for L, T in zip(LAYERS, tg)
    ]

    with tile.TileContext(nc) as tc:
        with (
            tc.tile_pool(name="const", bufs=1) as constp,
            tc.tile_pool(name="psum", bufs=2, space="PSUM") as psump,
            tc.tile_pool(name="g16", bufs=int(os.environ.get("KERNEL_G16B", "10"))) as gatep,
            tc.tile_pool(name="h32", bufs=int(os.environ.get("KERNEL_H32B", "6"))) as h32p,
            tc.tile_pool(name="h16", bufs=int(os.environ.get("KERNEL_H16B", "14"))) as h16p,
            tc.tile_pool(name="outp", bufs=2) as outp,
        ):
            # ---- load constants.  Weight tiles per input K-chunk; the tail
            # K-chunk is loaded replicated at each stack's partition offset so
            # lhsT/rhs SBUF base partitions match (PE row-group requirement).
            x0_t = constp.tile([9, b_core], MM_DT)
            nc.sync.dma_start(x0_t[:], x0_d[:])

            def load_w(dram, K, F, geo, name):
                tiles = []
                for (lo, hi) in _full_chunks(K) if K >= 128 else [(0, K)]:
                    t = constp.tile([hi - lo, F], MM_DT, name=f"{name}_{lo}")
                    nc.sync.dma_start(t[:], dram[lo:hi, :])
                    tiles.append(t)
                tail = None
                if K >= 128 and K % 128:
                    r, stride, s = geo["r"], geo["stride"], geo["s"]
                    assert K % 128 == r
                    tail = constp.tile([128, F], MM_DT, name=f"{name}_tail")
                    for k in range(s if STACK_TAILS else 1):
                        nc.sync.dma_start(
                            tail[k * stride:k * stride + r, :], dram[K - r:K, :])
                return tiles, tail

            W, BIA = {}, {}
            for li, L in enumerate(LAYERS, start=1):
                for g in ("u", "o"):
                    W[(li, g)] = load_w(wd_d[f"{li}{g}"], L["K"], L["F"],
                                        in_geo[li - 1], f"w{li}{g}")
                    bt_ = constp.tile([128, L["nf"] + 1], F32, name=f"b{li}{g}")
                    nc.sync.dma_start(bt_[:], bd_d[f"{li}{g}"][:])
                    BIA[(li, g)] = bt_
            WM, BM = {}, {}
            if MERGED_TAILS:
                for li, (L, T) in enumerate(zip(LAYERS, TAILG), start=1):
                    r, oo = T["r"], T["oo"]
                    wcols = oo + r
                    tiles = []
                    kcs = (_full_chunks(L["K"]) if L["K"] >= 128
                           else [(0, L["K"])])
                    for (lo, hi) in kcs:
                        t = constp.tile([hi - lo, wcols], MM_DT,
                                        name=f"wm{li}_{lo}")
                        if oo != r:
                            nc.gpsimd.memset(t[:], 0.0)
                        nc.sync.dma_start(
                            t[:, 0:r], wd_d[f"{li}u"][lo:hi, L["F"] - r:])
                        nc.sync.dma_start(
                            t[:, oo:oo + r], wd_d[f"{li}o"][lo:hi, L["F"] - r:])
                        tiles.append(t)
                    tailw = None
                    if L["K"] >= 128 and L["K"] % 128:
                        pg = in_geo[li - 1]
                        tailw = constp.tile([128, wcols], MM_DT,
                                            name=f"wm{li}_tail")
                        if oo != r:
                            nc.gpsimd.memset(tailw[:], 0.0)
                        for k in range(pg["s"]):
                            o = k * pg["stride"]
                            nc.sync.dma_start(
                                tailw[o:o + pg["r"], 0:r],
                                wd_d[f"{li}u"][L["K"] - pg["r"]:, L["F"] - r:])
                            nc.sync.dma_start(
                                tailw[o:o + pg["r"], oo:oo + r],
                                wd_d[f"{li}o"][L["K"] - pg["r"]:, L["F"] - r:])
                    WM[li] = (tiles, tailw)
                    bmt = constp.tile([128, 1], F32, name=f"bm{li}")
                    nc.sync.dma_start(bmt[:], bm_d[li][:])
                    BM[li] = bmt
                    selt = constp.tile([128, T["r"]], MM_DT, name=f"sel{li}")
                    nc.sync.dma_start(selt[:], bm_d[(li, "sel")][:])
                    BM[(li, "sel")] = selt
            WDt = load_w(wdd, DENSE_K, DENSE_F, in_geo[3], "wd")
            BD = constp.tile([128, 1], F32, name="bd")
            nc.sync.dma_start(BD[:], bdd[:])

            # ---- emission helpers --------------------------------
            def mk_rhs_l1(c0):
                def _rhs(kc, j):
                    return x0_t[0:9, c0 + j * SUB:c0 + (j + 1) * SUB], 0
                return _rhs

            def mk_rhs(full, tail, nf_, r_, stride_, s_):
                cps_ = BT // s_ if STACK_TAILS else BT

                def _rhs(kc, j):
                    if kc < nf_:
                        return full[kc][:, j * SUB:(j + 1) * SUB], 0
                    if STACK_TAILS:
                        k = (j * SUB) // cps_
                        col = (j * SUB) % cps_
                        return (tail[k * stride_:k * stride_ + r_,
                                     col:col + SUB], k * stride_)
                    return tail[0:r_, j * SUB:(j + 1) * SUB], 0
                return _rhs

            def emit_layer(li, c0, rhs, geo, ms=None, state=None):
                """One gate layer of big-tile at batch column c0; ms selects
                a subset of chunk indices (default: all full chunks then the
                tail).  Returns (h16_full list, h16_tail), accumulated in
                `state` across partial calls."""
                L = LAYERS[li - 1]
                nf, r, stride, s = L["nf"], L["r"], L["stride"], L["s"]
                n_full = len(geo["full_rows"])
                if state is None:
                    state = {"full": [None] * nf, "tail": None}
                h16_full, h16_tail = state["full"], state["tail"]
                for m in (list(range(nf)) + [nf]) if ms is None else ms:
                    is_tail = m == nf
                    fcols = (slice(L["F"] - r, L["F"]) if is_tail
                             else slice(m * 128, (m + 1) * 128))
                    g16 = {}
                    for g, func in (("u", AF.Sigmoid), ("o", AF.Tanh)):
                        wf, wt = W[(li, g)]
                        if not is_tail:
                            ps = psump.tile([128, BT], F32, tag="ps")
                            regions = [(0, 128, 0, j, j * SUB)
                                       for j in range(BT // SUB)]
                        elif STACK_TAILS:
                            cps = BT // s
                            ps = psump.tile([128, cps], F32, tag="ps")
                            regions = []
                            for k in range(s):
                                for h in range(cps // SUB):
                                    j = (k * cps) // SUB + h
                                    regions.append((k * stride, r, k, j, h * SUB))
                        else:
                            ps = psump.tile([r, BT], F32, tag="ps")
                            regions = [(0, r, 0, j, j * SUB)
                                       for j in range(BT // SUB)]
                        # Full K-chunks first: the tail K-chunk is the
                        # *last* thing the previous layer produces.
                        kc_order = (list(range(n_full))
                                    + ([n_full] if geo["r"] else []))
                        # Loop order vs has_written safety:
                        #  - full chunks: every region writes the same
                        #    partitions to a *disjoint* bank, so kc-outer /
                        #    region-inner is safe under both the per-partition
                        #    sim model and whole-bank bit clears -- and it
                        #    reuses the stationary weights across the 4
                        #    regions (4x fewer LDWEIGHTS on hardware; the
                        #    cost model does not charge LDWEIGHTS at all).
                        #  - stacked tails: regions share banks at different
                        #    partition offsets; each region's accumulation
                        #    group must complete before the next region
                        #    starts, so keep region-outer / kc-inner there.
                        if not is_tail or not STACK_TAILS:
                            for kc in kc_order:
                                rr0 = None
                                for (plo, psz, kstk, j, coff) in regions:
                                    rr, rbase = rhs(kc, j)
                                    if kc < n_full:
                                        lhs = wf[kc][0:geo["full_rows"][kc], fcols]
                                    else:
                                        lhs = wt[rbase:rbase + geo["r"], fcols]
                                    nc.tensor.matmul(
                                        ps[plo:plo + psz, coff:coff + SUB],
                                        lhs, rr,
                                        start=(kc == kc_order[0]),
                                        stop=(kc == kc_order[-1]),
                                        tile_position=(rbase, plo),
                                        skip_group_check=True)
                        else:
                            for (plo, psz, kstk, j, coff) in regions:
                                for kc in kc_order:
                                    rr, rbase = rhs(kc, j)
                                    if kc < n_full:
                                        lhs = wf[kc][0:geo["full_rows"][kc], fcols]
                                    else:
                                        lhs = wt[rbase:rbase + geo["r"], fcols]
                                    nc.tensor.matmul(
                                        ps[plo:plo + psz, coff:coff + SUB],
                                        lhs, rr,
                                        start=(kc == kc_order[0]),
                                        stop=(kc == kc_order[-1]),
                                        tile_position=(rbase, plo),
                                        skip_group_check=True)
                        gt = gatep.tile(list(ps.shape), MM_DT, tag="g16")
                        bcol = slice(nf, nf + 1) if is_tail else slice(m, m + 1)
                        if is_tail and STACK_TAILS and (
                                r != stride or SPLIT_TAILS):
                            for k in range(s):
                                sl = slice(k * stride, k * stride + r)
                                nc.scalar.activation(
                                    gt[sl, :], ps[sl, :], func,
                                    bias=BIA[(li, g)][sl, bcol])
                        else:
                            bias = BIA[(li, g)][0:ps.shape[0], bcol]
                            nc.scalar.activation(gt[:], ps[:], func, bias=bias)
                        g16[g] = gt
                    shape = list(g16["u"].shape)
                    h32 = h32p.tile(shape, F32, tag="h32")
                    h16 = h16p.tile(shape, MM_DT, tag="h16")
                    # h16 (the next layer's input, the latency-critical one)
                    # is produced FIRST as a direct fp16-out multiply; the
                    # fp32 product for the DRAM store follows off the critical
                    # path.  Both are the same DVE fp32-internal product, so
                    # h16 == cast(h32) exactly.
                    if is_tail and STACK_TAILS and (r != stride or SPLIT_TAILS):
                        for k in range(s):
                            sl = slice(k * stride, k * stride + r)
                            nc.vector.tensor_mul(h16[sl, :], g16["o"][sl, :],
                                                 g16["u"][sl, :])
                        for k in range(s):
                            sl = slice(k * stride, k * stride + r)
                            nc.vector.tensor_mul(h32[sl, :], g16["o"][sl, :],
                                                 g16["u"][sl, :])
                    else:
                        nc.vector.tensor_mul(h16[:], g16["o"][:], g16["u"][:])
                        nc.vector.tensor_mul(h32[:], g16["o"][:], g16["u"][:])
                    if not is_tail:
                        nc.gpsimd.dma_start(
                            x_out[li - 1][m * 128:(m + 1) * 128, c0:c0 + BT],
                            h32[:])
                        h16_full[m] = h16
                    elif STACK_TAILS:
                        cps = BT // s
                        for k in range(s):
                            nc.gpsimd.dma_start(
                                x_out[li - 1][L["F"] - r:L["F"],
                                              c0 + k * cps:c0 + (k + 1) * cps],
                                h32[k * stride:k * stride + r, :])
                        h16_tail = h16
                        state["tail"] = h16
                    else:
                        nc.gpsimd.dma_start(
                            x_out[li - 1][L["F"] - r:L["F"], c0:c0 + BT],
                            h32[:])
                        h16_tail = h16
                state["full"], state["tail"] = h16_full, h16_tail
                return h16_full, h16_tail

            def emit_merged_tail(li, c0, rhs, geo, state):
                """Both gates' tail features (r each) in ONE M-chunk of 2r
                rows, batch-stacked s2-fold: halves the tail matmul passes.
                Layout per stack k: rows [k*st2, k*st2+r) = u-part,
                [k*st2+r, k*st2+2r) = o-part.  The product needs u and o at
                the same partition base, which DVE requires, so the o-part is
                realigned with a small SBUF->SBUF DMA (Pool engine) first."""
                L, T = LAYERS[li - 1], TAILG[li - 1]
                r, st2, s2, oo = T["r"], T["stride"], T["s"], T["oo"]
                cps = BT // s2
                n_full = len(geo["full_rows"])
                wf, wt = WM[li]
                ps = psump.tile([128, cps], F32, tag="ps")
                regions = []
                for k in range(s2):
                    for h in range(cps // SUB):
                        j = (k * cps) // SUB + h
                        regions.append((k * st2, j, h * SUB))
                kc_order = list(range(n_full)) + ([n_full] if geo["r"] else [])

                def mm(plo, j, coff, kc):
                    rr, rbase = rhs(kc, j)
                    if kc < n_full:
                        lhs = wf[kc][0:geo["full_rows"][kc], 0:oo + r]
                    else:
                        lhs = wt[rbase:rbase + geo["r"], 0:oo + r]
                    nc.tensor.matmul(
                        ps[plo:plo + oo + r, coff:coff + SUB], lhs, rr,
                        start=(kc == kc_order[0]), stop=(kc == kc_order[-1]),
                        tile_position=(rbase, plo), skip_group_check=True)

                if s2 == 1:
                    # regions are disjoint banks on the same partitions:
                    # kc-outer is safe and reuses the stationary weights.
                    for kc in kc_order:
                        for (plo, j, coff) in regions:
                            mm(plo, j, coff, kc)
                else:
                    # stacks share banks at different partitions: complete
                    # each region's accumulation group before the next.
                    for (plo, j, coff) in regions:
                        for kc in kc_order:
                            mm(plo, j, coff, kc)

                gt = gatep.tile([128, cps], MM_DT, tag="g16")
                for k in range(s2):
                    o = k * st2
                    nc.scalar.activation(gt[o:o + r, :], ps[o:o + r, :],
                                         AF.Sigmoid, bias=BM[li][o:o + r, 0:1])
                    nc.scalar.activation(gt[o + oo:o + oo + r, :],
                                         ps[o + oo:o + oo + r, :],
                                         AF.Tanh,
                                         bias=BM[li][o + oo:o + oo + r, 0:1])
                selt = BM[(li, "sel")]

                def finish():
                    # PE realign: one-hot matmul moves each stack's tanh'd
                    # o-part into the (dead, already-ACT'd) u-rows of the
                    # tail psum tile, so the product is base-aligned for DVE.
                    for k in range(s2):
                        o = k * st2
                        for h in range(cps // SUB):
                            nc.tensor.matmul(
                                ps[o:o + r, h * SUB:(h + 1) * SUB],
                                selt[o + oo:o + oo + r, 0:r],
                                gt[o + oo:o + oo + r, h * SUB:(h + 1) * SUB],
                                start=True, stop=True,
                                tile_position=(o + oo, o),
                                skip_group_check=True)
                    h16 = h16p.tile([128, cps], MM_DT, tag="h16")
                    h32 = h32p.tile([128, cps], F32, tag="h32")
                    for k in range(s2):
                        o = k * st2
                        nc.vector.tensor_mul(h16[o:o + r, :], gt[o:o + r, :],
                                             ps[o:o + r, :])
                    for k in range(s2):
                        o = k * st2
                        nc.vector.tensor_mul(h32[o:o + r, :], gt[o:o + r, :],
                                             ps[o:o + r, :])
                    for k in range(s2):
                        nc.gpsimd.dma_start(
                            x_out[li - 1][L["F"] - r:L["F"],
                                          c0 + k * cps:c0 + (k + 1) * cps],
                            h32[k * st2:k * st2 + r, :])
                    state["tail"] = h16
                return finish

            def emit_gate_layer(li, c0, rhs, geo):
                nf = LAYERS[li - 1]["nf"]
                if MERGED_TAILS:
                    # tail MMs+ACTs first; the realign+product lands after
                    # the first full chunk so the ACT(o) -> realign dep never
                    # head-of-line-blocks the PE queue.
                    st = {"full": [None] * nf, "tail": None}
                    fin = emit_merged_tail(li, c0, rhs, geo, st)
                    emit_layer(li, c0, rhs, geo, ms=[0], state=st)
                    fin()
                    if nf > 1:
                        emit_layer(li, c0, rhs, geo,
                                   ms=list(range(1, nf)), state=st)
                    return st["full"], st["tail"]
                # tail group second (after full chunk 0): its ACT->mul chain
                # completes while the remaining full chunks run, so the next
                # layer's tail K-chunk is ready when consumers reach it.
                return emit_layer(li, c0, rhs, geo,
                                  ms=[0, nf] + list(range(1, nf)))

            def emit_dense(c0, rhs, geo):
                n_full = len(geo["full_rows"])
                wf, wt = WDt
                if STACK_TAILS:
                    ps = psump.tile([128, SUB], F32, tag="ps")
                    dj = [(j * 32, j, 0) for j in range(BT // SUB)]
                else:
                    ps = psump.tile([DENSE_F, BT], F32, tag="ps")
                    dj = [(0, j, j * SUB) for j in range(BT // SUB)]
                kc_order = list(range(n_full)) + [n_full]
                for (plo, j, coff) in dj:
                    for kc in kc_order:
                        rr, rbase = rhs(kc, j)
                        if kc < n_full:
                            lhs = wf[kc][:, 0:DENSE_F]
                        else:
                            lhs = wt[rbase:rbase + geo["r"], 0:DENSE_F]
                        nc.tensor.matmul(
                            ps[plo:plo + DENSE_F, coff:coff + SUB], lhs, rr,
                            start=(kc == kc_order[0]), stop=(kc == kc_order[-1]),
                            tile_position=(rbase, plo),
                            skip_group_check=True)
                o32 = outp.tile(list(ps.shape), F32, tag="o32")
                if STACK_TAILS:
                    for (plo, j, coff) in dj:
                        nc.vector.tensor_scalar_add(
                            o32[plo:plo + DENSE_F, :], ps[plo:plo + DENSE_F, :],
                            BD[plo:plo + DENSE_F, 0:1])
                    for (plo, j, coff) in dj:
                        nc.sync.dma_start(
                            y_d[0:DENSE_F, c0 + j * SUB:c0 + (j + 1) * SUB],
                            o32[plo:plo + DENSE_F, :])
                else:
                    nc.vector.tensor_scalar_add(o32[:], ps[:], BD[0:DENSE_F, 0:1])
                    nc.sync.dma_start(y_d[0:DENSE_F, c0:c0 + BT], o32[:])

            # ---- skewed software pipeline: L1 runs one big-tile ahead, so
            # its (independent-of-everything) matmuls fill the PE stalls at
            # the dense layer barriers of the previous big-tile.
            l1_out = {}
            SCHED = os.environ.get("KERNEL_SCHED", "A")
            NF1 = LAYERS[0]["nf"]

            def l1_pieces(w):
                c0 = w * BT
                rhs1 = mk_rhs_l1(c0)
                st = {"full": [None] * NF1, "tail": None}
                l1_out[w] = st
                ps = [
                    (lambda m=m: emit_layer(1, c0, rhs1, in_geo[0],
                                            ms=[m], state=st))
                    for m in range(NF1)
                ]
                if MERGED_TAILS:
                    fin_box = {}

                    def tail_mm():
                        fin_box["f"] = emit_merged_tail(1, c0, rhs1,
                                                        in_geo[0], st)

                    ps.insert(0, tail_mm)
                    ps.insert(2, lambda: fin_box["f"]())
                else:
                    ps.insert(1, lambda: emit_layer(1, c0, rhs1, in_geo[0],
                                                    ms=[NF1], state=st))
                return ps

            def emit_rest(b, pieces=()):
                # The next big-tile's L1 chunk-groups are emitted right after
                # each layer of this big-tile: their priority then sits
                # exactly at the layer-barrier stalls, giving PE independent
                # fill work while the barrier's ACT->mul->cast chain drains.
                c0 = b * BT
                L1, L2, L3 = LAYERS
                T1, T2, T3 = tg
                st = l1_out.pop(b)
                rhs = mk_rhs(st["full"], st["tail"],
                             L1["nf"], T1["r"], T1["stride"], T1["s"])
                hf, ht = emit_gate_layer(2, c0, rhs, in_geo[1])
                if len(pieces) > 0:
                    pieces[0]()
                rhs = mk_rhs(hf, ht, L2["nf"], T2["r"], T2["stride"], T2["s"])
                hf, ht = emit_gate_layer(3, c0, rhs, in_geo[2])
                if len(pieces) > 1:
                    pieces[1]()
                rhs = mk_rhs(hf, ht, L3["nf"], T3["r"], T3["stride"], T3["s"])
                emit_dense(c0, rhs, in_geo[3])
                for p in pieces[2:]:
                    p()

            if SCHED == "I":
                for w in range(nbt + 1):
                    pieces = l1_pieces(w) if w < nbt else ()
                    if w == 0:
                        for p in pieces:
                            p()
                    else:
                        emit_rest(w - 1, pieces)
            elif SCHED == "A2":
                # L1 runs TWO big-tiles ahead: twice the independent fill
                # inventory at the dense layer barriers.
                for ww in range(min(2, nbt)):
                    for p in l1_pieces(ww):
                        p()
                for b in range(nbt):
                    if b + 2 < nbt:
                        for p in l1_pieces(b + 2):
                            p()
                    emit_rest(b)
            else:
                for w in range(nbt + 1):
                    if w < nbt:
                        pieces = l1_pieces(w)
                        for p in pieces:
                            p()
                    if w >= 1:
                        emit_rest(w - 1)

    nc.compile()
    return nc


_NC_CACHE = {}


def _get_nc(b_core):
    if b_core not in _NC_CACHE:
        _NC_CACHE[b_core] = build_bass(b_core)
    return _NC_CACHE[b_core]


# ---------------------------------------------------------------- entry point
def kernel(**inputs):
    consts = _prep_consts(inputs)
    x = np.asarray(inputs["inputs"], np.float32).reshape(B_TOTAL, 9)

    in_maps = []
    for c in range(N_CORES):
        m = dict(consts)
        xc = x[c * B_CORE:(c + 1) * B_CORE]
        m["x0"] = np.ascontiguousarray(xc.T).astype(MM_NP)
        in_maps.append(m)

    nc = _get_nc(B_CORE)
    trace = bool(int(os.environ.get("KERNEL_TRACE", "0")))
    res = run_bass_kernel_spmd(nc, in_maps, core_ids=list(range(N_CORES)),
                               trace=trace)
    if trace and res.exec_time_ns is not None:
        print(f"HW exec time: {res.exec_time_ns} ns")
        kernel.last_exec_time_ns = res.exec_time_ns

    outs, x1s, x2s, x3s = [], [], [], []
    for c in range(N_CORES):
        r = res.results[c]
        outs.append(np.ascontiguousarray(r["y"].T))
        x1s.append(np.ascontiguousarray(r["x1"].T).reshape(B_CORE, 32, 3, 3))
        x2s.append(np.ascontiguousarray(r["x2"].T).reshape(B_CORE, 64, 3, 3))
        x3s.append(np.ascontiguousarray(r["x3"].T).reshape(B_CORE, 16, 3, 3))
    return (np.concatenate(outs), np.concatenate(x1s),
            np.concatenate(x2s), np.concatenate(x3s))


# revision 39
# speedup vs baseline: 1.0097x; 1.0097x over previous
"""Trainium2 Bass kernel for nn_CRNN: 3 stacked ConvGRU cells (applied once,
zero initial hidden state) + dense head, B=65536 samples of [1,3,3].

Math: with h=0 the GRU cell reduces to
    x_out = tanh(conv3(x, Wo[:, :cin]) + bo) * sigmoid(conv3(x, Wu[:, :cin]) + bu)
(the reset gate is dead).  A 3x3 SAME conv on a 3x3 image is a dense linear map
on the flattened [cin*9] feature vector, so the whole net is a 4-layer MLP over
features 9 -> 288 -> 576 -> 144 -> 9 with gate products between layers.

Kernel layout: features on partitions, batch on the free dim.  Pure data
parallel over 8 cores (8192 samples each).  Matmuls run in fp16 (FP22
multiply, fp32 PSUM accumulate), activations on ScalarE straight out of PSUM,
gate product on VectorE (fp32 result stored, fp16 copy feeds the next layer).

Outputs are stored [features, batch] contiguous; the host transposes on
unshard (that IS the unshard gather).
"""

import os
import sys

import numpy as np

sys.path.insert(0, "/opt/trn_rl_repo")

import concourse.bass as bass
import concourse.mybir as mybir
import concourse.tile as tile
from concourse import bacc
from concourse.bass_utils import run_bass_kernel_spmd

# ---------------------------------------------------------------- config
N_CORES = 8
B_TOTAL = 65536
B_CORE = B_TOTAL // N_CORES
BT = 2048          # big batch tile (free-dim) per pipeline step
SUB = 512          # matmul free-dim (= one PSUM bank of fp32)
MM_DT = mybir.dt.float16    # matmul operand dtype
MM_NP = np.float16
STACK_TAILS = bool(int(os.environ.get("KERNEL_STACK_TAILS", "1")))
SPLIT_TAILS = bool(int(os.environ.get("KERNEL_SPLIT_TAILS", "0")))

F32 = mybir.dt.float32
AF = mybir.ActivationFunctionType

# layer geometry: K input feats, F gate width, nf = F//128 full chunks,
# r = F%128 tail rows, stride = 32-aligned stack stride, s = stacks.
LAYERS = [
    dict(K=9, F=288, nf=2, r=32, stride=32, s=4),
    dict(K=288, F=576, nf=4, r=64, stride=64, s=2),
    dict(K=576, F=144, nf=1, r=16, stride=32, s=4),
]
MERGED_TAILS = bool(int(os.environ.get("KERNEL_MERGED_TAILS", "0")))
# merged-tail layout per layer: blocks of [u(r); o(r)] stacked s2-fold at
# stride2 partitions, covering cps = BT // s2 batch columns per stack.
TAILG = [
    dict(r=32, stride=64, s=2, oo=32),    # L1: [u32|o32]       -> [128, 1024]
    dict(r=64, stride=128, s=1, oo=64),   # L2: [u64|o64]       -> [128, 2048]
    dict(r=16, stride=64, s=2, oo=32),    # L3: [u16|pad16|o16] -> [128, 1024]
]
DENSE_K, DENSE_F = 144, 9
CH = [32, 64, 16]
CIN = [1, 32, 64]


# ---------------------------------------------------------------- host-side prep
def _conv_to_dense(w, cin_used):
    """w: [Cout, Cin, 3, 3] OIHW SAME conv on 3x3 images -> A: [Cout*9, cin_used*9]
    with y_flat = A @ x_flat, flat feature index = c*9 + i*3 + j."""
    w = np.asarray(w, np.float32)[:, :cin_used]
    cout, cin = w.shape[0], w.shape[1]
    A = np.zeros((cout, 9, cin, 9), np.float32)
    for i in range(3):
        for j in range(3):
            for di in range(3):
                for dj in range(3):
                    si, sj = i + di - 1, j + dj - 1
                    if 0 <= si < 3 and 0 <= sj < 3:
                        A[:, i * 3 + j, :, si * 3 + sj] = w[:, :, di, dj]
    return A.reshape(cout * 9, cin * 9)


def _bias_pack(b_chan, L):
    """Per-channel bias -> [128, nf+1] fp32: col m = features m*128..m*128+127,
    last col = tail features replicated at each stack's partition offset."""
    nf, r, stride, s = L["nf"], L["r"], L["stride"], L["s"]
    bf = np.repeat(np.asarray(b_chan, np.float32), 9)
    out = np.zeros((128, nf + 1), np.float32)
    for m in range(nf):
        out[:, m] = bf[m * 128:(m + 1) * 128]
    for k in range(s if STACK_TAILS else 1):
        out[k * stride:k * stride + r, nf] = bf[L["F"] - r:]
    return out


def _prep_consts(inputs):
    c = {}
    for li, L in enumerate(LAYERS, start=1):
        for g in ("u", "o"):
            A = _conv_to_dense(inputs[f"w{li}{g}"], CIN[li - 1])      # [F, K]
            c[f"w{li}{g}"] = np.ascontiguousarray(A.T).astype(MM_NP)  # [K, F]
            c[f"b{li}{g}"] = _bias_pack(inputs[f"b{li}{g}"], LAYERS[li - 1])
    c["wd"] = np.ascontiguousarray(
        np.asarray(inputs["wd"], np.float32).T).astype(MM_NP)          # [144, 9]
    bd = np.zeros((128, 1), np.float32)
    for k in range(4 if STACK_TAILS else 1):
        bd[k * 32:k * 32 + DENSE_F, 0] = np.asarray(inputs["bd"], np.float32)
    c["bd"] = bd
    if MERGED_TAILS:
        for li, (L, T) in enumerate(zip(LAYERS, TAILG), start=1):
            r, st2, s2 = T["r"], T["stride"], T["s"]
            bu = np.repeat(np.asarray(inputs[f"b{li}u"], np.float32), 9)[L["F"] - r:]
            bo = np.repeat(np.asarray(inputs[f"b{li}o"], np.float32), 9)[L["F"] - r:]
            oo = T["oo"]
            bm = np.zeros((128, 1), np.float32)
            for k in range(s2):
                bm[k * st2:k * st2 + r, 0] = bu
                bm[k * st2 + oo:k * st2 + oo + r, 0] = bo
            c[f"bm{li}"] = bm
            sel = np.zeros((128, r), MM_NP)
            for k in range(s2):
                for i in range(r):
                    sel[k * st2 + oo + i, i] = 1.0
            c[f"sel{li}"] = sel
    return c


def _full_chunks(K):
    """[(lo, hi), ...] covering the full-128 part of K."""
    return [(m * 128, (m + 1) * 128) for m in range(K // 128)]


# ---------------------------------------------------------------- bass kernel
def build_bass(b_core=B_CORE):
    nc = bacc.Bacc("TRN2", target_bir_lowering=False, debug=False)
    nbt = b_core // BT
    assert b_core % BT == 0

    # ---- DRAM tensors
    x0_d = nc.dram_tensor("x0", [9, b_core], MM_DT, kind="ExternalInput").ap()
    wd_d, bd_d = {}, {}
    for li, L in enumerate(LAYERS, start=1):
        for g in ("u", "o"):
            wd_d[f"{li}{g}"] = nc.dram_tensor(
                f"w{li}{g}", [L["K"], L["F"]], MM_DT, kind="ExternalInput").ap()
            bd_d[f"{li}{g}"] = nc.dram_tensor(
                f"b{li}{g}", [128, L["nf"] + 1], F32, kind="ExternalInput").ap()
    wdd = nc.dram_tensor("wd", [DENSE_K, DENSE_F], MM_DT, kind="ExternalInput").ap()
    bdd = nc.dram_tensor("bd", [128, 1], F32, kind="ExternalInput").ap()
    bm_d = {}
    if MERGED_TAILS:
        for li in (1, 2, 3):
            bm_d[li] = nc.dram_tensor(
                f"bm{li}", [128, 1], F32, kind="ExternalInput").ap()
            bm_d[(li, "sel")] = nc.dram_tensor(
                f"sel{li}", [128, TAILG[li - 1]["r"]], MM_DT,
                kind="ExternalInput").ap()
    x_out = [
        nc.dram_tensor(f"x{li}", [L["F"], b_core], F32, kind="ExternalOutput").ap()
        for li, L in enumerate(LAYERS, start=1)
    ]
    y_d = nc.dram_tensor("y", [DENSE_F, b_core], F32, kind="ExternalOutput").ap()

    # geometry of each matmul-input source, in order: L1 in, L2 in, L3 in, dense in
    tg = TAILG if MERGED_TAILS else [
        dict(r=L["r"], stride=L["stride"], s=L["s"]) for L in LAYERS]
    in_geo = [dict(full_rows=[9], r=0, stride=0, s=0)] + [
        dict(full_rows=[128] * L["nf"], r=T["r"], stride=T["stride"], s=T["s"])
        for L, T in zip(LAYERS, tg)
    ]

    with tile.TileContext(nc) as tc:
        with (
            tc.tile_pool(name="const", bufs=1) as constp,
            tc.tile_pool(name="psum", bufs=2, space="PSUM") as psump,
            tc.tile_pool(name="g16", bufs=int(os.environ.get("KERNEL_G16B", "10"))) as gatep,
            tc.tile_pool(name="h32", bufs=int(os.environ.get("KERNEL_H32B", "6"))) as h32p,
            tc.tile_pool(name="h16", bufs=int(os.environ.get("KERNEL_H16B", "14"))) as h16p,
            tc.tile_pool(name="outp", bufs=2) as outp,
        ):
            # ---- load constants.  Weight tiles per input K-chunk; the tail
            # K-chunk is loaded replicated at each stack's partition offset so
            # lhsT/rhs SBUF base partitions match (PE row-group requirement).
            x0_t = constp.tile([9, b_core], MM_DT)
            nc.sync.dma_start(x0_t[:], x0_d[:])

            def load_w(dram, K, F, geo, name):
                tiles = []
                for (lo, hi) in _full_chunks(K) if K >= 128 else [(0, K)]:
                    t = constp.tile([hi - lo, F], MM_DT, name=f"{name}_{lo}")
                    nc.sync.dma_start(t[:], dram[lo:hi, :])
                    tiles.append(t)
                tail = None
                if K >= 128 and K % 128:
                    r, stride, s = geo["r"], geo["stride"], geo["s"]
                    assert K % 128 == r
                    tail = constp.tile([128, F], MM_DT, name=f"{name}_tail")
                    for k in range(s if STACK_TAILS else 1):
                        nc.sync.dma_start(
                            tail[k * stride:k * stride + r, :], dram[K - r:K, :])
                return tiles, tail

            W, BIA = {}, {}
            for li, L in enumerate(LAYERS, start=1):
                for g in ("u", "o"):
                    W[(li, g)] = load_w(wd_d[f"{li}{g}"], L["K"], L["F"],
                                        in_geo[li - 1], f"w{li}{g}")
                    bt_ = constp.tile([128, L["nf"] + 1], F32, name=f"b{li}{g}")
                    nc.sync.dma_start(bt_[:], bd_d[f"{li}{g}"][:])
                    BIA[(li, g)] = bt_
            WM, BM = {}, {}
            if MERGED_TAILS:
                for li, (L, T) in enumerate(zip(LAYERS, TAILG), start=1):
                    r, oo = T["r"], T["oo"]
                    wcols = oo + r
                    tiles = []
                    kcs = (_full_chunks(L["K"]) if L["K"] >= 128
                           else [(0, L["K"])])
                    for (lo, hi) in kcs:
                        t = constp.tile([hi - lo, wcols], MM_DT,
                                        name=f"wm{li}_{lo}")
                        if oo != r:
                            nc.gpsimd.memset(t[:], 0.0)
                        nc.sync.dma_start(
                            t[:, 0:r], wd_d[f"{li}u"][lo:hi, L["F"] - r:])
                        nc.sync.dma_start(
                            t[:, oo:oo + r], wd_d[f"{li}o"][lo:hi, L["F"] - r:])
                        tiles.append(t)
                    tailw = None
                    if L["K"] >= 128 and L["K"] % 128:
                        pg = in_geo[li - 1]
                        tailw = constp.tile([128, wcols], MM_DT,
                                            name=f"wm{li}_tail")
                        if oo != r:
                            nc.gpsimd.memset(tailw[:], 0.0)
                        for k in range(pg["s"]):
                            o = k * pg["stride"]
                            nc.sync.dma_start(
                                tailw[o:o + pg["r"], 0:r],
                                wd_d[f"{li}u"][L["K"] - pg["r"]:, L["F"] - r:])
                            nc.sync.dma_start(
                                tailw[o:o + pg["r"], oo:oo + r],
                                wd_d[f"{li}o"][L["K"] - pg["r"]:, L["F"] - r:])
                    WM[li] = (tiles, tailw)
                    bmt = constp.tile([128, 1], F32, name=f"bm{li}")
                    nc.sync.dma_start(bmt[:], bm_d[li][:])
                    BM[li] = bmt
                    selt = constp.tile([128, T["r"]], MM_DT, name=f"sel{li}")
                    nc.sync.dma_start(selt[:], bm_d[(li, "sel")][:])
                    BM[(li, "sel")] = selt
            WDt = load_w(wdd, DENSE_K, DENSE_F, in_geo[3], "wd")
            BD = constp.tile([128, 1], F32, name="bd")
            nc.sync.dma_start(BD[:], bdd[:])

            # ---- emission helpers --------------------------------
            def mk_rhs_l1(c0):
                def _rhs(kc, j):
                    return x0_t[0:9, c0 + j * SUB:c0 + (j + 1) * SUB], 0
                return _rhs

            def mk_rhs(full, tail, nf_, r_, stride_, s_):
                cps_ = BT // s_ if STACK_TAILS else BT

                def _rhs(kc, j):
                    if kc < nf_:
                        return full[kc][:, j * SUB:(j + 1) * SUB], 0
                    if STACK_TAILS:
                        k = (j * SUB) // cps_
                        col = (j * SUB) % cps_
                        return (tail[k * stride_:k * stride_ + r_,
                                     col:col + SUB], k * stride_)
                    return tail[0:r_, j * SUB:(j + 1) * SUB], 0
                return _rhs

            def emit_layer(li, c0, rhs, geo, ms=None, state=None):
                """One gate layer of big-tile at batch column c0; ms selects
                a subset of chunk indices (default: all full chunks then the
                tail).  Returns (h16_full list, h16_tail), accumulated in
                `state` across partial calls."""
                L = LAYERS[li - 1]
                nf, r, stride, s = L["nf"], L["r"], L["stride"], L["s"]
                n_full = len(geo["full_rows"])
                if state is None:
                    state = {"full": [None] * nf, "tail": None}
                h16_full, h16_tail = state["full"], state["tail"]
                for m in (list(range(nf)) + [nf]) if ms is None else ms:
                    is_tail = m == nf
                    fcols = (slice(L["F"] - r, L["F"]) if is_tail
                             else slice(m * 128, (m + 1) * 128))
                    g16 = {}
                    for g, func in (("u", AF.Sigmoid), ("o", AF.Tanh)):
                        wf, wt = W[(li, g)]
                        if not is_tail:
                            ps = psump.tile([128, BT], F32, tag="ps")
                            regions = [(0, 128, 0, j, j * SUB)
                                       for j in range(BT // SUB)]
                        elif STACK_TAILS:
                            cps = BT // s
                            ps = psump.tile([128, cps], F32, tag="ps")
                            if r != stride:
                                # zero the gap rows the matmuls never touch so
                                # the full-width ACT/mul read defined data
                                nc.vector.memset(ps[:], 0.0)
                            regions = []
                            for k in range(s):
                                for h in range(cps // SUB):
                                    j = (k * cps) // SUB + h
                                    regions.append((k * stride, r, k, j, h * SUB))
                        else:
                            ps = psump.tile([r, BT], F32, tag="ps")
                            regions = [(0, r, 0, j, j * SUB)
                                       for j in range(BT // SUB)]
                        # Full K-chunks first: the tail K-chunk is the
                        # *last* thing the previous layer produces.
                        kc_order = (list(range(n_full))
                                    + ([n_full] if geo["r"] else []))
                        # Loop order vs has_written safety:
                        #  - full chunks: every region writes the same
                        #    partitions to a *disjoint* bank, so kc-outer /
                        #    region-inner is safe under both the per-partition
                        #    sim model and whole-bank bit clears -- and it
                        #    reuses the stationary weights across the 4
                        #    regions (4x fewer LDWEIGHTS on hardware; the
                        #    cost model does not charge LDWEIGHTS at all).
                        #  - stacked tails: regions share banks at different
                        #    partition offsets; each region's accumulation
                        #    group must complete before the next region
                        #    starts, so keep region-outer / kc-inner there.
                        if not is_tail or not STACK_TAILS:
                            for kc in kc_order:
                                rr0 = None
                                for (plo, psz, kstk, j, coff) in regions:
                                    rr, rbase = rhs(kc, j)
                                    if kc < n_full:
                                        lhs = wf[kc][0:geo["full_rows"][kc], fcols]
                                    else:
                                        lhs = wt[rbase:rbase + geo["r"], fcols]
                                    nc.tensor.matmul(
                                        ps[plo:plo + psz, coff:coff + SUB],
                                        lhs, rr,
                                        start=(kc == kc_order[0]),
                                        stop=(kc == kc_order[-1]),
                                        tile_position=(rbase, plo),
                                        skip_group_check=True)
                        else:
                            for (plo, psz, kstk, j, coff) in regions:
                                for kc in kc_order:
                                    rr, rbase = rhs(kc, j)
                                    if kc < n_full:
                                        lhs = wf[kc][0:geo["full_rows"][kc], fcols]
                                    else:
                                        lhs = wt[rbase:rbase + geo["r"], fcols]
                                    nc.tensor.matmul(
                                        ps[plo:plo + psz, coff:coff + SUB],
                                        lhs, rr,
                                        start=(kc == kc_order[0]),
                                        stop=(kc == kc_order[-1]),
                                        tile_position=(rbase, plo),
                                        skip_group_check=True)
                        gt = gatep.tile(list(ps.shape), MM_DT, tag="g16")
                        bcol = slice(nf, nf + 1) if is_tail else slice(m, m + 1)
                        if is_tail and STACK_TAILS and SPLIT_TAILS:
                            for k in range(s):
                                sl = slice(k * stride, k * stride + r)
                                nc.scalar.activation(
                                    gt[sl, :], ps[sl, :], func,
                                    bias=BIA[(li, g)][sl, bcol])
                        else:
                            bias = BIA[(li, g)][0:ps.shape[0], bcol]
                            nc.scalar.activation(gt[:], ps[:], func, bias=bias)
                        g16[g] = gt
                    shape = list(g16["u"].shape)
                    h32 = h32p.tile(shape, F32, tag="h32")
                    h16 = h16p.tile(shape, MM_DT, tag="h16")
                    # h16 (the next layer's input, the latency-critical one)
                    # is produced FIRST as a direct fp16-out multiply; the
                    # fp32 product for the DRAM store follows off the critical
                    # path.  Both are the same DVE fp32-internal product, so
                    # h16 == cast(h32) exactly.
                    # Full-width even when r != stride: the unwritten gap
                    # rows carry junk that nothing reads (stores and the next
                    # layer's rhs slice valid rows only); one 128-lane
                    # instruction replaces s narrow ones.
                    if is_tail and STACK_TAILS and SPLIT_TAILS:
                        for k in range(s):
                            sl = slice(k * stride, k * stride + r)
                            nc.vector.tensor_mul(h16[sl, :], g16["o"][sl, :],
                                                 g16["u"][sl, :])
                        for k in range(s):
                            sl = slice(k * stride, k * stride + r)
                            nc.vector.tensor_mul(h32[sl, :], g16["o"][sl, :],
                                                 g16["u"][sl, :])
                    else:
                        nc.vector.tensor_mul(h16[:], g16["o"][:], g16["u"][:])
                        nc.vector.tensor_mul(h32[:], g16["o"][:], g16["u"][:])
                    if not is_tail:
                        nc.gpsimd.dma_start(
                            x_out[li - 1][m * 128:(m + 1) * 128, c0:c0 + BT],
                            h32[:])
                        h16_full[m] = h16
                    elif STACK_TAILS:
                        cps = BT // s
                        for k in range(s):
                            nc.gpsimd.dma_start(
                                x_out[li - 1][L["F"] - r:L["F"],
                                              c0 + k * cps:c0 + (k + 1) * cps],
                                h32[k * stride:k * stride + r, :])
                        h16_tail = h16
                        state["tail"] = h16
                    else:
                        nc.gpsimd.dma_start(
                            x_out[li - 1][L["F"] - r:L["F"], c0:c0 + BT],
                            h32[:])
                        h16_tail = h16
                state["full"], state["tail"] = h16_full, h16_tail
                return h16_full, h16_tail

            def emit_merged_tail(li, c0, rhs, geo, state):
                """Both gates' tail features (r each) in ONE M-chunk of 2r
                rows, batch-stacked s2-fold: halves the tail matmul passes.
                Layout per stack k: rows [k*st2, k*st2+r) = u-part,
                [k*st2+r, k*st2+2r) = o-part.  The product needs u and o at
                the same partition base, which DVE requires, so the o-part is
                realigned with a small SBUF->SBUF DMA (Pool engine) first."""
                L, T = LAYERS[li - 1], TAILG[li - 1]
                r, st2, s2, oo = T["r"], T["stride"], T["s"], T["oo"]
                cps = BT // s2
                n_full = len(geo["full_rows"])
                wf, wt = WM[li]
                ps = psump.tile([128, cps], F32, tag="ps")
                regions = []
                for k in range(s2):
                    for h in range(cps // SUB):
                        j = (k * cps) // SUB + h
                        regions.append((k * st2, j, h * SUB))
                kc_order = list(range(n_full)) + ([n_full] if geo["r"] else [])

                def mm(plo, j, coff, kc):
                    rr, rbase = rhs(kc, j)
                    if kc < n_full:
                        lhs = wf[kc][0:geo["full_rows"][kc], 0:oo + r]
                    else:
                        lhs = wt[rbase:rbase + geo["r"], 0:oo + r]
                    nc.tensor.matmul(
                        ps[plo:plo + oo + r, coff:coff + SUB], lhs, rr,
                        start=(kc == kc_order[0]), stop=(kc == kc_order[-1]),
                        tile_position=(rbase, plo), skip_group_check=True)

                if s2 == 1:
                    # regions are disjoint banks on the same partitions:
                    # kc-outer is safe and reuses the stationary weights.
                    for kc in kc_order:
                        for (plo, j, coff) in regions:
                            mm(plo, j, coff, kc)
                else:
                    # stacks share banks at different partitions: complete
                    # each region's accumulation group before the next.
                    for (plo, j, coff) in regions:
                        for kc in kc_order:
                            mm(plo, j, coff, kc)

                gt = gatep.tile([128, cps], MM_DT, tag="g16")
                for k in range(s2):
                    o = k * st2
                    nc.scalar.activation(gt[o:o + r, :], ps[o:o + r, :],
                                         AF.Sigmoid, bias=BM[li][o:o + r, 0:1])
                    nc.scalar.activation(gt[o + oo:o + oo + r, :],
                                         ps[o + oo:o + oo + r, :],
                                         AF.Tanh,
                                         bias=BM[li][o + oo:o + oo + r, 0:1])
                selt = BM[(li, "sel")]

                def finish():
                    # PE realign: one-hot matmul moves each stack's tanh'd
                    # o-part into the (dead, already-ACT'd) u-rows of the
                    # tail psum tile, so the product is base-aligned for DVE.
                    for k in range(s2):
                        o = k * st2
                        for h in range(cps // SUB):
                            nc.tensor.matmul(
                                ps[o:o + r, h * SUB:(h + 1) * SUB],
                                selt[o + oo:o + oo + r, 0:r],
                                gt[o + oo:o + oo + r, h * SUB:(h + 1) * SUB],
                                start=True, stop=True,
                                tile_position=(o + oo, o),
                                skip_group_check=True)
                    h16 = h16p.tile([128, cps], MM_DT, tag="h16")
                    h32 = h32p.tile([128, cps], F32, tag="h32")
                    for k in range(s2):
                        o = k * st2
                        nc.vector.tensor_mul(h16[o:o + r, :], gt[o:o + r, :],
                                             ps[o:o + r, :])
                    for k in range(s2):
                        o = k * st2
                        nc.vector.tensor_mul(h32[o:o + r, :], gt[o:o + r, :],
                                             ps[o:o + r, :])
                    for k in range(s2):
                        nc.gpsimd.dma_start(
                            x_out[li - 1][L["F"] - r:L["F"],
                                          c0 + k * cps:c0 + (k + 1) * cps],
                            h32[k * st2:k * st2 + r, :])
                    state["tail"] = h16
                return finish

            def emit_gate_layer(li, c0, rhs, geo):
                nf = LAYERS[li - 1]["nf"]
                if MERGED_TAILS:
                    # tail MMs+ACTs first; the realign+product lands after
                    # the first full chunk so the ACT(o) -> realign dep never
                    # head-of-line-blocks the PE queue.
                    st = {"full": [None] * nf, "tail": None}
                    fin = emit_merged_tail(li, c0, rhs, geo, st)
                    emit_layer(li, c0, rhs, geo, ms=[0], state=st)
                    fin()
                    if nf > 1:
                        emit_layer(li, c0, rhs, geo,
                                   ms=list(range(1, nf)), state=st)
                    return st["full"], st["tail"]
                # tail group second (after full chunk 0): its ACT->mul chain
                # completes while the remaining full chunks run, so the next
                # layer's tail K-chunk is ready when consumers reach it.
                return emit_layer(li, c0, rhs, geo,
                                  ms=[0, nf] + list(range(1, nf)))

            def emit_dense(c0, rhs, geo):
                n_full = len(geo["full_rows"])
                wf, wt = WDt
                if STACK_TAILS:
                    ps = psump.tile([128, SUB], F32, tag="ps")
                    nc.vector.memset(ps[:], 0.0)
                    dj = [(j * 32, j, 0) for j in range(BT // SUB)]
                else:
                    ps = psump.tile([DENSE_F, BT], F32, tag="ps")
                    dj = [(0, j, j * SUB) for j in range(BT // SUB)]
                kc_order = list(range(n_full)) + [n_full]
                for (plo, j, coff) in dj:
                    for kc in kc_order:
                        rr, rbase = rhs(kc, j)
                        if kc < n_full:
                            lhs = wf[kc][:, 0:DENSE_F]
                        else:
                            lhs = wt[rbase:rbase + geo["r"], 0:DENSE_F]
                        nc.tensor.matmul(
                            ps[plo:plo + DENSE_F, coff:coff + SUB], lhs, rr,
                            start=(kc == kc_order[0]), stop=(kc == kc_order[-1]),
                            tile_position=(rbase, plo),
                            skip_group_check=True)
                o32 = outp.tile(list(ps.shape), F32, tag="o32")
                if STACK_TAILS:
                    nc.vector.tensor_scalar_add(o32[:], ps[:], BD[:, 0:1])
                    for (plo, j, coff) in dj:
                        nc.sync.dma_start(
                            y_d[0:DENSE_F, c0 + j * SUB:c0 + (j + 1) * SUB],
                            o32[plo:plo + DENSE_F, :])
                else:
                    nc.vector.tensor_scalar_add(o32[:], ps[:], BD[0:DENSE_F, 0:1])
                    nc.sync.dma_start(y_d[0:DENSE_F, c0:c0 + BT], o32[:])

            # ---- skewed software pipeline: L1 runs one big-tile ahead, so
            # its (independent-of-everything) matmuls fill the PE stalls at
            # the dense layer barriers of the previous big-tile.
            l1_out = {}
            SCHED = os.environ.get("KERNEL_SCHED", "A")
            NF1 = LAYERS[0]["nf"]

            def l1_pieces(w):
                c0 = w * BT
                rhs1 = mk_rhs_l1(c0)
                st = {"full": [None] * NF1, "tail": None}
                l1_out[w] = st
                ps = [
                    (lambda m=m: emit_layer(1, c0, rhs1, in_geo[0],
                                            ms=[m], state=st))
                    for m in range(NF1)
                ]
                if MERGED_TAILS:
                    fin_box = {}

                    def tail_mm():
                        fin_box["f"] = emit_merged_tail(1, c0, rhs1,
                                                        in_geo[0], st)

                    ps.insert(0, tail_mm)
                    ps.insert(2, lambda: fin_box["f"]())
                else:
                    ps.insert(1, lambda: emit_layer(1, c0, rhs1, in_geo[0],
                                                    ms=[NF1], state=st))
                return ps

            def emit_rest(b, pieces=()):
                # The next big-tile's L1 chunk-groups are emitted right after
                # each layer of this big-tile: their priority then sits
                # exactly at the layer-barrier stalls, giving PE independent
                # fill work while the barrier's ACT->mul->cast chain drains.
                c0 = b * BT
                L1, L2, L3 = LAYERS
                T1, T2, T3 = tg
                st = l1_out.pop(b)
                rhs = mk_rhs(st["full"], st["tail"],
                             L1["nf"], T1["r"], T1["stride"], T1["s"])
                hf, ht = emit_gate_layer(2, c0, rhs, in_geo[1])
                if len(pieces) > 0:
                    pieces[0]()
                rhs = mk_rhs(hf, ht, L2["nf"], T2["r"], T2["stride"], T2["s"])
                hf, ht = emit_gate_layer(3, c0, rhs, in_geo[2])
                if len(pieces) > 1:
                    pieces[1]()
                rhs = mk_rhs(hf, ht, L3["nf"], T3["r"], T3["stride"], T3["s"])
                emit_dense(c0, rhs, in_geo[3])
                for p in pieces[2:]:
                    p()

            if SCHED == "I":
                for w in range(nbt + 1):
                    pieces = l1_pieces(w) if w < nbt else ()
                    if w == 0:
                        for p in pieces:
                            p()
                    else:
                        emit_rest(w - 1, pieces)
            elif SCHED == "A2":
                # L1 runs TWO big-tiles ahead: twice the independent fill
                # inventory at the dense layer barriers.
                for ww in range(min(2, nbt)):
                    for p in l1_pieces(ww):
                        p()
                for b in range(nbt):
                    if b + 2 < nbt:
                        for p in l1_pieces(b + 2):
                            p()
                    emit_rest(b)
            else:
                for w in range(nbt + 1):
                    if w < nbt:
                        pieces = l1_pieces(w)
                        for p in pieces:
                            p()
                    if w >= 1:
                        emit_rest(w - 1)

    nc.compile()
    return nc


_NC_CACHE = {}


def _get_nc(b_core):
    if b_core not in _NC_CACHE:
        _NC_CACHE[b_core] = build_bass(b_core)
    return _NC_CACHE[b_core]


# ---------------------------------------------------------------- entry point
def kernel(**inputs):
    consts = _prep_consts(inputs)
    x = np.asarray(inputs["inputs"], np.float32).reshape(B_TOTAL, 9)

    in_maps = []
    for c in range(N_CORES):
        m = dict(consts)
        xc = x[c * B_CORE:(c + 1) * B_CORE]
        m["x0"] = np.ascontiguousarray(xc.T).astype(MM_NP)
        in_maps.append(m)

    nc = _get_nc(B_CORE)
    trace = bool(int(os.environ.get("KERNEL_TRACE", "0")))
    res = run_bass_kernel_spmd(nc, in_maps, core_ids=list(range(N_CORES)),
                               trace=trace)
    if trace and res.exec_time_ns is not None:
        print(f"HW exec time: {res.exec_time_ns} ns")
        kernel.last_exec_time_ns = res.exec_time_ns

    outs, x1s, x2s, x3s = [], [], [], []
    for c in range(N_CORES):
        r = res.results[c]
        outs.append(np.ascontiguousarray(r["y"].T))
        x1s.append(np.ascontiguousarray(r["x1"].T).reshape(B_CORE, 32, 3, 3))
        x2s.append(np.ascontiguousarray(r["x2"].T).reshape(B_CORE, 64, 3, 3))
        x3s.append(np.ascontiguousarray(r["x3"].T).reshape(B_CORE, 16, 3, 3))
    return (np.concatenate(outs), np.concatenate(x1s),
            np.concatenate(x2s), np.concatenate(x3s))


# revision 40
# speedup vs baseline: 1.0468x; 1.0368x over previous
"""Trainium2 Bass kernel for nn_CRNN: 3 stacked ConvGRU cells (applied once,
zero initial hidden state) + dense head, B=65536 samples of [1,3,3].

Math: with h=0 the GRU cell reduces to
    x_out = tanh(conv3(x, Wo[:, :cin]) + bo) * sigmoid(conv3(x, Wu[:, :cin]) + bu)
(the reset gate is dead).  A 3x3 SAME conv on a 3x3 image is a dense linear map
on the flattened [cin*9] feature vector, so the whole net is a 4-layer MLP over
features 9 -> 288 -> 576 -> 144 -> 9 with gate products between layers.

Kernel layout: features on partitions, batch on the free dim.  Pure data
parallel over 8 cores (8192 samples each).  Matmuls run in fp16 (FP22
multiply, fp32 PSUM accumulate), activations on ScalarE straight out of PSUM,
gate product on VectorE (fp32 result stored, fp16 copy feeds the next layer).

Outputs are stored [features, batch] contiguous; the host transposes on
unshard (that IS the unshard gather).
"""

import os
import sys

import numpy as np

sys.path.insert(0, "/opt/trn_rl_repo")

import concourse.bass as bass
import concourse.mybir as mybir
import concourse.tile as tile
from concourse import bacc
from concourse.bass_utils import run_bass_kernel_spmd

# ---------------------------------------------------------------- config
N_CORES = 8
B_TOTAL = 65536
B_CORE = B_TOTAL // N_CORES
BT = 2048          # big batch tile (free-dim) per pipeline step
SUB = 512          # matmul free-dim (= one PSUM bank of fp32)
MM_DT = mybir.dt.float16    # matmul operand dtype
MM_NP = np.float16
STACK_TAILS = bool(int(os.environ.get("KERNEL_STACK_TAILS", "1")))
SPLIT_TAILS = bool(int(os.environ.get("KERNEL_SPLIT_TAILS", "0")))
# Zero-fill the never-consumed gap rows of stacked-tail/dense PSUM tiles.
# Only needed to satisfy the CoreSim race detector: on hardware the reads
# are ordered after the previous slot occupant completes (Tile WAR release)
# and the junk rows are never stored or consumed.
SIM_SAFE = bool(int(os.environ.get("KERNEL_SIM_SAFE", "0")))

F32 = mybir.dt.float32
AF = mybir.ActivationFunctionType

# layer geometry: K input feats, F gate width, nf = F//128 full chunks,
# r = F%128 tail rows, stride = 32-aligned stack stride, s = stacks.
LAYERS = [
    dict(K=9, F=288, nf=2, r=32, stride=32, s=4),
    dict(K=288, F=576, nf=4, r=64, stride=64, s=2),
    dict(K=576, F=144, nf=1, r=16, stride=32, s=4),
]
MERGED_TAILS = bool(int(os.environ.get("KERNEL_MERGED_TAILS", "0")))
# merged-tail layout per layer: blocks of [u(r); o(r)] stacked s2-fold at
# stride2 partitions, covering cps = BT // s2 batch columns per stack.
TAILG = [
    dict(r=32, stride=64, s=2, oo=32),    # L1: [u32|o32]       -> [128, 1024]
    dict(r=64, stride=128, s=1, oo=64),   # L2: [u64|o64]       -> [128, 2048]
    dict(r=16, stride=64, s=2, oo=32),    # L3: [u16|pad16|o16] -> [128, 1024]
]
DENSE_K, DENSE_F = 144, 9
CH = [32, 64, 16]
CIN = [1, 32, 64]


# ---------------------------------------------------------------- host-side prep
def _conv_to_dense(w, cin_used):
    """w: [Cout, Cin, 3, 3] OIHW SAME conv on 3x3 images -> A: [Cout*9, cin_used*9]
    with y_flat = A @ x_flat, flat feature index = c*9 + i*3 + j."""
    w = np.asarray(w, np.float32)[:, :cin_used]
    cout, cin = w.shape[0], w.shape[1]
    A = np.zeros((cout, 9, cin, 9), np.float32)
    for i in range(3):
        for j in range(3):
            for di in range(3):
                for dj in range(3):
                    si, sj = i + di - 1, j + dj - 1
                    if 0 <= si < 3 and 0 <= sj < 3:
                        A[:, i * 3 + j, :, si * 3 + sj] = w[:, :, di, dj]
    return A.reshape(cout * 9, cin * 9)


def _bias_pack(b_chan, L):
    """Per-channel bias -> [128, nf+1] fp32: col m = features m*128..m*128+127,
    last col = tail features replicated at each stack's partition offset."""
    nf, r, stride, s = L["nf"], L["r"], L["stride"], L["s"]
    bf = np.repeat(np.asarray(b_chan, np.float32), 9)
    out = np.zeros((128, nf + 1), np.float32)
    for m in range(nf):
        out[:, m] = bf[m * 128:(m + 1) * 128]
    for k in range(s if STACK_TAILS else 1):
        out[k * stride:k * stride + r, nf] = bf[L["F"] - r:]
    return out


def _prep_consts(inputs):
    c = {}
    for li, L in enumerate(LAYERS, start=1):
        for g in ("u", "o"):
            A = _conv_to_dense(inputs[f"w{li}{g}"], CIN[li - 1])      # [F, K]
            c[f"w{li}{g}"] = np.ascontiguousarray(A.T).astype(MM_NP)  # [K, F]
            c[f"b{li}{g}"] = _bias_pack(inputs[f"b{li}{g}"], LAYERS[li - 1])
    c["wd"] = np.ascontiguousarray(
        np.asarray(inputs["wd"], np.float32).T).astype(MM_NP)          # [144, 9]
    bd = np.zeros((128, 1), np.float32)
    for k in range(4 if STACK_TAILS else 1):
        bd[k * 32:k * 32 + DENSE_F, 0] = np.asarray(inputs["bd"], np.float32)
    c["bd"] = bd
    if MERGED_TAILS:
        for li, (L, T) in enumerate(zip(LAYERS, TAILG), start=1):
            r, st2, s2 = T["r"], T["stride"], T["s"]
            bu = np.repeat(np.asarray(inputs[f"b{li}u"], np.float32), 9)[L["F"] - r:]
            bo = np.repeat(np.asarray(inputs[f"b{li}o"], np.float32), 9)[L["F"] - r:]
            oo = T["oo"]
            bm = np.zeros((128, 1), np.float32)
            for k in range(s2):
                bm[k * st2:k * st2 + r, 0] = bu
                bm[k * st2 + oo:k * st2 + oo + r, 0] = bo
            c[f"bm{li}"] = bm
            sel = np.zeros((128, r), MM_NP)
            for k in range(s2):
                for i in range(r):
                    sel[k * st2 + oo + i, i] = 1.0
            c[f"sel{li}"] = sel
    return c


def _full_chunks(K):
    """[(lo, hi), ...] covering the full-128 part of K."""
    return [(m * 128, (m + 1) * 128) for m in range(K // 128)]


# ---------------------------------------------------------------- bass kernel
def build_bass(b_core=B_CORE):
    nc = bacc.Bacc("TRN2", target_bir_lowering=False, debug=False)
    nbt = b_core // BT
    assert b_core % BT == 0

    # ---- DRAM tensors
    x0_d = nc.dram_tensor("x0", [9, b_core], MM_DT, kind="ExternalInput").ap()
    wd_d, bd_d = {}, {}
    for li, L in enumerate(LAYERS, start=1):
        for g in ("u", "o"):
            wd_d[f"{li}{g}"] = nc.dram_tensor(
                f"w{li}{g}", [L["K"], L["F"]], MM_DT, kind="ExternalInput").ap()
            bd_d[f"{li}{g}"] = nc.dram_tensor(
                f"b{li}{g}", [128, L["nf"] + 1], F32, kind="ExternalInput").ap()
    wdd = nc.dram_tensor("wd", [DENSE_K, DENSE_F], MM_DT, kind="ExternalInput").ap()
    bdd = nc.dram_tensor("bd", [128, 1], F32, kind="ExternalInput").ap()
    bm_d = {}
    if MERGED_TAILS:
        for li in (1, 2, 3):
            bm_d[li] = nc.dram_tensor(
                f"bm{li}", [128, 1], F32, kind="ExternalInput").ap()
            bm_d[(li, "sel")] = nc.dram_tensor(
                f"sel{li}", [128, TAILG[li - 1]["r"]], MM_DT,
                kind="ExternalInput").ap()
    x_out = [
        nc.dram_tensor(f"x{li}", [L["F"], b_core], F32, kind="ExternalOutput").ap()
        for li, L in enumerate(LAYERS, start=1)
    ]
    y_d = nc.dram_tensor("y", [DENSE_F, b_core], F32, kind="ExternalOutput").ap()

    # geometry of each matmul-input source, in order: L1 in, L2 in, L3 in, dense in
    tg = TAILG if MERGED_TAILS else [
        dict(r=L["r"], stride=L["stride"], s=L["s"]) for L in LAYERS]
    in_geo = [dict(full_rows=[9], r=0, stride=0, s=0)] + [
        dict(full_rows=[128] * L["nf"], r=T["r"], stride=T["stride"], s=T["s"])
        for L, T in zip(LAYERS, tg)
    ]

    with tile.TileContext(nc) as tc:
        with (
            tc.tile_pool(name="const", bufs=1) as constp,
            tc.tile_pool(name="psum", bufs=2, space="PSUM") as psump,
            tc.tile_pool(name="g16", bufs=int(os.environ.get("KERNEL_G16B", "10"))) as gatep,
            tc.tile_pool(name="h32", bufs=int(os.environ.get("KERNEL_H32B", "6"))) as h32p,
            tc.tile_pool(name="h16", bufs=int(os.environ.get("KERNEL_H16B", "14"))) as h16p,
            tc.tile_pool(name="outp", bufs=2) as outp,
        ):
            # ---- load constants.  Weight tiles per input K-chunk; the tail
            # K-chunk is loaded replicated at each stack's partition offset so
            # lhsT/rhs SBUF base partitions match (PE row-group requirement).
            x0_t = constp.tile([9, b_core], MM_DT)
            nc.sync.dma_start(x0_t[:], x0_d[:])

            def load_w(dram, K, F, geo, name):
                tiles = []
                for (lo, hi) in _full_chunks(K) if K >= 128 else [(0, K)]:
                    t = constp.tile([hi - lo, F], MM_DT, name=f"{name}_{lo}")
                    nc.sync.dma_start(t[:], dram[lo:hi, :])
                    tiles.append(t)
                tail = None
                if K >= 128 and K % 128:
                    r, stride, s = geo["r"], geo["stride"], geo["s"]
                    assert K % 128 == r
                    tail = constp.tile([128, F], MM_DT, name=f"{name}_tail")
                    for k in range(s if STACK_TAILS else 1):
                        nc.sync.dma_start(
                            tail[k * stride:k * stride + r, :], dram[K - r:K, :])
                return tiles, tail

            W, BIA = {}, {}
            for li, L in enumerate(LAYERS, start=1):
                for g in ("u", "o"):
                    W[(li, g)] = load_w(wd_d[f"{li}{g}"], L["K"], L["F"],
                                        in_geo[li - 1], f"w{li}{g}")
                    bt_ = constp.tile([128, L["nf"] + 1], F32, name=f"b{li}{g}")
                    nc.sync.dma_start(bt_[:], bd_d[f"{li}{g}"][:])
                    BIA[(li, g)] = bt_
            WM, BM = {}, {}
            if MERGED_TAILS:
                for li, (L, T) in enumerate(zip(LAYERS, TAILG), start=1):
                    r, oo = T["r"], T["oo"]
                    wcols = oo + r
                    tiles = []
                    kcs = (_full_chunks(L["K"]) if L["K"] >= 128
                           else [(0, L["K"])])
                    for (lo, hi) in kcs:
                        t = constp.tile([hi - lo, wcols], MM_DT,
                                        name=f"wm{li}_{lo}")
                        if oo != r:
                            nc.gpsimd.memset(t[:], 0.0)
                        nc.sync.dma_start(
                            t[:, 0:r], wd_d[f"{li}u"][lo:hi, L["F"] - r:])
                        nc.sync.dma_start(
                            t[:, oo:oo + r], wd_d[f"{li}o"][lo:hi, L["F"] - r:])
                        tiles.append(t)
                    tailw = None
                    if L["K"] >= 128 and L["K"] % 128:
                        pg = in_geo[li - 1]
                        tailw = constp.tile([128, wcols], MM_DT,
                                            name=f"wm{li}_tail")
                        if oo != r:
                            nc.gpsimd.memset(tailw[:], 0.0)
                        for k in range(pg["s"]):
                            o = k * pg["stride"]
                            nc.sync.dma_start(
                                tailw[o:o + pg["r"], 0:r],
                                wd_d[f"{li}u"][L["K"] - pg["r"]:, L["F"] - r:])
                            nc.sync.dma_start(
                                tailw[o:o + pg["r"], oo:oo + r],
                                wd_d[f"{li}o"][L["K"] - pg["r"]:, L["F"] - r:])
                    WM[li] = (tiles, tailw)
                    bmt = constp.tile([128, 1], F32, name=f"bm{li}")
                    nc.sync.dma_start(bmt[:], bm_d[li][:])
                    BM[li] = bmt
                    selt = constp.tile([128, T["r"]], MM_DT, name=f"sel{li}")
                    nc.sync.dma_start(selt[:], bm_d[(li, "sel")][:])
                    BM[(li, "sel")] = selt
            WDt = load_w(wdd, DENSE_K, DENSE_F, in_geo[3], "wd")
            BD = constp.tile([128, 1], F32, name="bd")
            nc.sync.dma_start(BD[:], bdd[:])

            # ---- emission helpers --------------------------------
            def mk_rhs_l1(c0):
                def _rhs(kc, j):
                    return x0_t[0:9, c0 + j * SUB:c0 + (j + 1) * SUB], 0
                return _rhs

            def mk_rhs(full, tail, nf_, r_, stride_, s_):
                cps_ = BT // s_ if STACK_TAILS else BT

                def _rhs(kc, j):
                    if kc < nf_:
                        return full[kc][:, j * SUB:(j + 1) * SUB], 0
                    if STACK_TAILS:
                        k = (j * SUB) // cps_
                        col = (j * SUB) % cps_
                        return (tail[k * stride_:k * stride_ + r_,
                                     col:col + SUB], k * stride_)
                    return tail[0:r_, j * SUB:(j + 1) * SUB], 0
                return _rhs

            def emit_layer(li, c0, rhs, geo, ms=None, state=None):
                """One gate layer of big-tile at batch column c0; ms selects
                a subset of chunk indices (default: all full chunks then the
                tail).  Returns (h16_full list, h16_tail), accumulated in
                `state` across partial calls."""
                L = LAYERS[li - 1]
                nf, r, stride, s = L["nf"], L["r"], L["stride"], L["s"]
                n_full = len(geo["full_rows"])
                if state is None:
                    state = {"full": [None] * nf, "tail": None}
                h16_full, h16_tail = state["full"], state["tail"]
                for m in (list(range(nf)) + [nf]) if ms is None else ms:
                    is_tail = m == nf
                    fcols = (slice(L["F"] - r, L["F"]) if is_tail
                             else slice(m * 128, (m + 1) * 128))
                    g16 = {}
                    for g, func in (("u", AF.Sigmoid), ("o", AF.Tanh)):
                        wf, wt = W[(li, g)]
                        if not is_tail:
                            ps = psump.tile([128, BT], F32, tag="ps")
                            regions = [(0, 128, 0, j, j * SUB)
                                       for j in range(BT // SUB)]
                        elif STACK_TAILS:
                            cps = BT // s
                            ps = psump.tile([128, cps], F32, tag="ps")
                            if r != stride and SIM_SAFE:
                                nc.vector.memset(ps[:], 0.0)
                            regions = []
                            for k in range(s):
                                for h in range(cps // SUB):
                                    j = (k * cps) // SUB + h
                                    regions.append((k * stride, r, k, j, h * SUB))
                        else:
                            ps = psump.tile([r, BT], F32, tag="ps")
                            regions = [(0, r, 0, j, j * SUB)
                                       for j in range(BT // SUB)]
                        # Full K-chunks first: the tail K-chunk is the
                        # *last* thing the previous layer produces.
                        kc_order = (list(range(n_full))
                                    + ([n_full] if geo["r"] else []))
                        # Loop order vs has_written safety:
                        #  - full chunks: every region writes the same
                        #    partitions to a *disjoint* bank, so kc-outer /
                        #    region-inner is safe under both the per-partition
                        #    sim model and whole-bank bit clears -- and it
                        #    reuses the stationary weights across the 4
                        #    regions (4x fewer LDWEIGHTS on hardware; the
                        #    cost model does not charge LDWEIGHTS at all).
                        #  - stacked tails: regions share banks at different
                        #    partition offsets; each region's accumulation
                        #    group must complete before the next region
                        #    starts, so keep region-outer / kc-inner there.
                        if not is_tail or not STACK_TAILS:
                            for kc in kc_order:
                                rr0 = None
                                for (plo, psz, kstk, j, coff) in regions:
                                    rr, rbase = rhs(kc, j)
                                    if kc < n_full:
                                        lhs = wf[kc][0:geo["full_rows"][kc], fcols]
                                    else:
                                        lhs = wt[rbase:rbase + geo["r"], fcols]
                                    nc.tensor.matmul(
                                        ps[plo:plo + psz, coff:coff + SUB],
                                        lhs, rr,
                                        start=(kc == kc_order[0]),
                                        stop=(kc == kc_order[-1]),
                                        tile_position=(rbase, plo),
                                        skip_group_check=True)
                        else:
                            for (plo, psz, kstk, j, coff) in regions:
                                for kc in kc_order:
                                    rr, rbase = rhs(kc, j)
                                    if kc < n_full:
                                        lhs = wf[kc][0:geo["full_rows"][kc], fcols]
                                    else:
                                        lhs = wt[rbase:rbase + geo["r"], fcols]
                                    nc.tensor.matmul(
                                        ps[plo:plo + psz, coff:coff + SUB],
                                        lhs, rr,
                                        start=(kc == kc_order[0]),
                                        stop=(kc == kc_order[-1]),
                                        tile_position=(rbase, plo),
                                        skip_group_check=True)
                        gt = gatep.tile(list(ps.shape), MM_DT, tag="g16")
                        bcol = slice(nf, nf + 1) if is_tail else slice(m, m + 1)
                        if is_tail and STACK_TAILS and SPLIT_TAILS:
                            for k in range(s):
                                sl = slice(k * stride, k * stride + r)
                                nc.scalar.activation(
                                    gt[sl, :], ps[sl, :], func,
                                    bias=BIA[(li, g)][sl, bcol])
                        else:
                            bias = BIA[(li, g)][0:ps.shape[0], bcol]
                            nc.scalar.activation(gt[:], ps[:], func, bias=bias)
                        g16[g] = gt
                    shape = list(g16["u"].shape)
                    h32 = h32p.tile(shape, F32, tag="h32")
                    h16 = h16p.tile(shape, MM_DT, tag="h16")
                    # h16 (the next layer's input, the latency-critical one)
                    # is produced FIRST as a direct fp16-out multiply; the
                    # fp32 product for the DRAM store follows off the critical
                    # path.  Both are the same DVE fp32-internal product, so
                    # h16 == cast(h32) exactly.
                    # Full-width even when r != stride: the unwritten gap
                    # rows carry junk that nothing reads (stores and the next
                    # layer's rhs slice valid rows only); one 128-lane
                    # instruction replaces s narrow ones.
                    if is_tail and STACK_TAILS and SPLIT_TAILS:
                        for k in range(s):
                            sl = slice(k * stride, k * stride + r)
                            nc.vector.tensor_mul(h16[sl, :], g16["o"][sl, :],
                                                 g16["u"][sl, :])
                        for k in range(s):
                            sl = slice(k * stride, k * stride + r)
                            nc.vector.tensor_mul(h32[sl, :], g16["o"][sl, :],
                                                 g16["u"][sl, :])
                    else:
                        nc.vector.tensor_mul(h16[:], g16["o"][:], g16["u"][:])
                        nc.vector.tensor_mul(h32[:], g16["o"][:], g16["u"][:])
                    if not is_tail:
                        nc.gpsimd.dma_start(
                            x_out[li - 1][m * 128:(m + 1) * 128, c0:c0 + BT],
                            h32[:])
                        h16_full[m] = h16
                    elif STACK_TAILS:
                        cps = BT // s
                        for k in range(s):
                            nc.gpsimd.dma_start(
                                x_out[li - 1][L["F"] - r:L["F"],
                                              c0 + k * cps:c0 + (k + 1) * cps],
                                h32[k * stride:k * stride + r, :])
                        h16_tail = h16
                        state["tail"] = h16
                    else:
                        nc.gpsimd.dma_start(
                            x_out[li - 1][L["F"] - r:L["F"], c0:c0 + BT],
                            h32[:])
                        h16_tail = h16
                state["full"], state["tail"] = h16_full, h16_tail
                return h16_full, h16_tail

            def emit_merged_tail(li, c0, rhs, geo, state):
                """Both gates' tail features (r each) in ONE M-chunk of 2r
                rows, batch-stacked s2-fold: halves the tail matmul passes.
                Layout per stack k: rows [k*st2, k*st2+r) = u-part,
                [k*st2+r, k*st2+2r) = o-part.  The product needs u and o at
                the same partition base, which DVE requires, so the o-part is
                realigned with a small SBUF->SBUF DMA (Pool engine) first."""
                L, T = LAYERS[li - 1], TAILG[li - 1]
                r, st2, s2, oo = T["r"], T["stride"], T["s"], T["oo"]
                cps = BT // s2
                n_full = len(geo["full_rows"])
                wf, wt = WM[li]
                ps = psump.tile([128, cps], F32, tag="ps")
                regions = []
                for k in range(s2):
                    for h in range(cps // SUB):
                        j = (k * cps) // SUB + h
                        regions.append((k * st2, j, h * SUB))
                kc_order = list(range(n_full)) + ([n_full] if geo["r"] else [])

                def mm(plo, j, coff, kc):
                    rr, rbase = rhs(kc, j)
                    if kc < n_full:
                        lhs = wf[kc][0:geo["full_rows"][kc], 0:oo + r]
                    else:
                        lhs = wt[rbase:rbase + geo["r"], 0:oo + r]
                    nc.tensor.matmul(
                        ps[plo:plo + oo + r, coff:coff + SUB], lhs, rr,
                        start=(kc == kc_order[0]), stop=(kc == kc_order[-1]),
                        tile_position=(rbase, plo), skip_group_check=True)

                if s2 == 1:
                    # regions are disjoint banks on the same partitions:
                    # kc-outer is safe and reuses the stationary weights.
                    for kc in kc_order:
                        for (plo, j, coff) in regions:
                            mm(plo, j, coff, kc)
                else:
                    # stacks share banks at different partitions: complete
                    # each region's accumulation group before the next.
                    for (plo, j, coff) in regions:
                        for kc in kc_order:
                            mm(plo, j, coff, kc)

                gt = gatep.tile([128, cps], MM_DT, tag="g16")
                for k in range(s2):
                    o = k * st2
                    nc.scalar.activation(gt[o:o + r, :], ps[o:o + r, :],
                                         AF.Sigmoid, bias=BM[li][o:o + r, 0:1])
                    nc.scalar.activation(gt[o + oo:o + oo + r, :],
                                         ps[o + oo:o + oo + r, :],
                                         AF.Tanh,
                                         bias=BM[li][o + oo:o + oo + r, 0:1])
                selt = BM[(li, "sel")]

                def finish():
                    # PE realign: one-hot matmul moves each stack's tanh'd
                    # o-part into the (dead, already-ACT'd) u-rows of the
                    # tail psum tile, so the product is base-aligned for DVE.
                    for k in range(s2):
                        o = k * st2
                        for h in range(cps // SUB):
                            nc.tensor.matmul(
                                ps[o:o + r, h * SUB:(h + 1) * SUB],
                                selt[o + oo:o + oo + r, 0:r],
                                gt[o + oo:o + oo + r, h * SUB:(h + 1) * SUB],
                                start=True, stop=True,
                                tile_position=(o + oo, o),
                                skip_group_check=True)
                    h16 = h16p.tile([128, cps], MM_DT, tag="h16")
                    h32 = h32p.tile([128, cps], F32, tag="h32")
                    for k in range(s2):
                        o = k * st2
                        nc.vector.tensor_mul(h16[o:o + r, :], gt[o:o + r, :],
                                             ps[o:o + r, :])
                    for k in range(s2):
                        o = k * st2
                        nc.vector.tensor_mul(h32[o:o + r, :], gt[o:o + r, :],
                                             ps[o:o + r, :])
                    for k in range(s2):
                        nc.gpsimd.dma_start(
                            x_out[li - 1][L["F"] - r:L["F"],
                                          c0 + k * cps:c0 + (k + 1) * cps],
                            h32[k * st2:k * st2 + r, :])
                    state["tail"] = h16
                return finish

            def emit_gate_layer(li, c0, rhs, geo):
                nf = LAYERS[li - 1]["nf"]
                if MERGED_TAILS:
                    # tail MMs+ACTs first; the realign+product lands after
                    # the first full chunk so the ACT(o) -> realign dep never
                    # head-of-line-blocks the PE queue.
                    st = {"full": [None] * nf, "tail": None}
                    fin = emit_merged_tail(li, c0, rhs, geo, st)
                    emit_layer(li, c0, rhs, geo, ms=[0], state=st)
                    fin()
                    if nf > 1:
                        emit_layer(li, c0, rhs, geo,
                                   ms=list(range(1, nf)), state=st)
                    return st["full"], st["tail"]
                # tail group second (after full chunk 0): its ACT->mul chain
                # completes while the remaining full chunks run, so the next
                # layer's tail K-chunk is ready when consumers reach it.
                return emit_layer(li, c0, rhs, geo,
                                  ms=[0, nf] + list(range(1, nf)))

            def emit_dense(c0, rhs, geo):
                n_full = len(geo["full_rows"])
                wf, wt = WDt
                if STACK_TAILS:
                    ps = psump.tile([128, SUB], F32, tag="ps")
                    if SIM_SAFE:
                        nc.vector.memset(ps[:], 0.0)
                    dj = [(j * 32, j, 0) for j in range(BT // SUB)]
                else:
                    ps = psump.tile([DENSE_F, BT], F32, tag="ps")
                    dj = [(0, j, j * SUB) for j in range(BT // SUB)]
                kc_order = list(range(n_full)) + [n_full]
                for (plo, j, coff) in dj:
                    for kc in kc_order:
                        rr, rbase = rhs(kc, j)
                        if kc < n_full:
                            lhs = wf[kc][:, 0:DENSE_F]
                        else:
                            lhs = wt[rbase:rbase + geo["r"], 0:DENSE_F]
                        nc.tensor.matmul(
                            ps[plo:plo + DENSE_F, coff:coff + SUB], lhs, rr,
                            start=(kc == kc_order[0]), stop=(kc == kc_order[-1]),
                            tile_position=(rbase, plo),
                            skip_group_check=True)
                o32 = outp.tile(list(ps.shape), F32, tag="o32")
                if STACK_TAILS:
                    nc.vector.tensor_scalar_add(o32[:], ps[:], BD[:, 0:1])
                    for (plo, j, coff) in dj:
                        nc.sync.dma_start(
                            y_d[0:DENSE_F, c0 + j * SUB:c0 + (j + 1) * SUB],
                            o32[plo:plo + DENSE_F, :])
                else:
                    nc.vector.tensor_scalar_add(o32[:], ps[:], BD[0:DENSE_F, 0:1])
                    nc.sync.dma_start(y_d[0:DENSE_F, c0:c0 + BT], o32[:])

            # ---- skewed software pipeline: L1 runs one big-tile ahead, so
            # its (independent-of-everything) matmuls fill the PE stalls at
            # the dense layer barriers of the previous big-tile.
            l1_out = {}
            SCHED = os.environ.get("KERNEL_SCHED", "A")
            NF1 = LAYERS[0]["nf"]

            def l1_pieces(w):
                c0 = w * BT
                rhs1 = mk_rhs_l1(c0)
                st = {"full": [None] * NF1, "tail": None}
                l1_out[w] = st
                ps = [
                    (lambda m=m: emit_layer(1, c0, rhs1, in_geo[0],
                                            ms=[m], state=st))
                    for m in range(NF1)
                ]
                if MERGED_TAILS:
                    fin_box = {}

                    def tail_mm():
                        fin_box["f"] = emit_merged_tail(1, c0, rhs1,
                                                        in_geo[0], st)

                    ps.insert(0, tail_mm)
                    ps.insert(2, lambda: fin_box["f"]())
                else:
                    ps.insert(1, lambda: emit_layer(1, c0, rhs1, in_geo[0],
                                                    ms=[NF1], state=st))
                return ps

            def emit_rest(b, pieces=()):
                # The next big-tile's L1 chunk-groups are emitted right after
                # each layer of this big-tile: their priority then sits
                # exactly at the layer-barrier stalls, giving PE independent
                # fill work while the barrier's ACT->mul->cast chain drains.
                c0 = b * BT
                L1, L2, L3 = LAYERS
                T1, T2, T3 = tg
                st = l1_out.pop(b)
                rhs = mk_rhs(st["full"], st["tail"],
                             L1["nf"], T1["r"], T1["stride"], T1["s"])
                hf, ht = emit_gate_layer(2, c0, rhs, in_geo[1])
                if len(pieces) > 0:
                    pieces[0]()
                rhs = mk_rhs(hf, ht, L2["nf"], T2["r"], T2["stride"], T2["s"])
                hf, ht = emit_gate_layer(3, c0, rhs, in_geo[2])
                if len(pieces) > 1:
                    pieces[1]()
                rhs = mk_rhs(hf, ht, L3["nf"], T3["r"], T3["stride"], T3["s"])
                emit_dense(c0, rhs, in_geo[3])
                for p in pieces[2:]:
                    p()

            if SCHED == "I":
                for w in range(nbt + 1):
                    pieces = l1_pieces(w) if w < nbt else ()
                    if w == 0:
                        for p in pieces:
                            p()
                    else:
                        emit_rest(w - 1, pieces)
            elif SCHED == "A2":
                # L1 runs TWO big-tiles ahead: twice the independent fill
                # inventory at the dense layer barriers.
                for ww in range(min(2, nbt)):
                    for p in l1_pieces(ww):
                        p()
                for b in range(nbt):
                    if b + 2 < nbt:
                        for p in l1_pieces(b + 2):
                            p()
                    emit_rest(b)
            else:
                for w in range(nbt + 1):
                    if w < nbt:
                        pieces = l1_pieces(w)
                        for p in pieces:
                            p()
                    if w >= 1:
                        emit_rest(w - 1)

    nc.compile()
    return nc


_NC_CACHE = {}


def _get_nc(b_core):
    if b_core not in _NC_CACHE:
        _NC_CACHE[b_core] = build_bass(b_core)
    return _NC_CACHE[b_core]


# ---------------------------------------------------------------- entry point
def kernel(**inputs):
    consts = _prep_consts(inputs)
    x = np.asarray(inputs["inputs"], np.float32).reshape(B_TOTAL, 9)

    in_maps = []
    for c in range(N_CORES):
        m = dict(consts)
        xc = x[c * B_CORE:(c + 1) * B_CORE]
        m["x0"] = np.ascontiguousarray(xc.T).astype(MM_NP)
        in_maps.append(m)

    nc = _get_nc(B_CORE)
    trace = bool(int(os.environ.get("KERNEL_TRACE", "0")))
    res = run_bass_kernel_spmd(nc, in_maps, core_ids=list(range(N_CORES)),
                               trace=trace)
    if trace and res.exec_time_ns is not None:
        print(f"HW exec time: {res.exec_time_ns} ns")
        kernel.last_exec_time_ns = res.exec_time_ns

    outs, x1s, x2s, x3s = [], [], [], []
    for c in range(N_CORES):
        r = res.results[c]
        outs.append(np.ascontiguousarray(r["y"].T))
        x1s.append(np.ascontiguousarray(r["x1"].T).reshape(B_CORE, 32, 3, 3))
        x2s.append(np.ascontiguousarray(r["x2"].T).reshape(B_CORE, 64, 3, 3))
        x3s.append(np.ascontiguousarray(r["x3"].T).reshape(B_CORE, 16, 3, 3))
    return (np.concatenate(outs), np.concatenate(x1s),
            np.concatenate(x2s), np.concatenate(x3s))


# revision 41
# speedup vs baseline: 1.0483x; 1.0014x over previous
"""Trainium2 Bass kernel for nn_CRNN: 3 stacked ConvGRU cells (applied once,
zero initial hidden state) + dense head, B=65536 samples of [1,3,3].

Math: with h=0 the GRU cell reduces to
    x_out = tanh(conv3(x, Wo[:, :cin]) + bo) * sigmoid(conv3(x, Wu[:, :cin]) + bu)
(the reset gate is dead).  A 3x3 SAME conv on a 3x3 image is a dense linear map
on the flattened [cin*9] feature vector, so the whole net is a 4-layer MLP over
features 9 -> 288 -> 576 -> 144 -> 9 with gate products between layers.

Kernel layout: features on partitions, batch on the free dim.  Pure data
parallel over 8 cores (8192 samples each).  Matmuls run in fp16 (FP22
multiply, fp32 PSUM accumulate), activations on ScalarE straight out of PSUM,
gate product on VectorE (fp32 result stored, fp16 copy feeds the next layer).

Outputs are stored [features, batch] contiguous; the host transposes on
unshard (that IS the unshard gather).
"""

import os
import sys

import numpy as np

sys.path.insert(0, "/opt/trn_rl_repo")

import concourse.bass as bass
import concourse.mybir as mybir
import concourse.tile as tile
from concourse import bacc
from concourse.bass_utils import run_bass_kernel_spmd

# ---------------------------------------------------------------- config
N_CORES = 8
B_TOTAL = 65536
B_CORE = B_TOTAL // N_CORES
BT = 2048          # big batch tile (free-dim) per pipeline step
SUB = 512          # matmul free-dim (= one PSUM bank of fp32)
MM_DT = mybir.dt.float16    # matmul operand dtype
MM_NP = np.float16
STACK_TAILS = bool(int(os.environ.get("KERNEL_STACK_TAILS", "1")))
SPLIT_TAILS = bool(int(os.environ.get("KERNEL_SPLIT_TAILS", "0")))
# Zero-fill the never-consumed gap rows of stacked-tail/dense PSUM tiles.
# Only needed to satisfy the CoreSim race detector: on hardware the reads
# are ordered after the previous slot occupant completes (Tile WAR release)
# and the junk rows are never stored or consumed.
SIM_SAFE = bool(int(os.environ.get("KERNEL_SIM_SAFE", "0")))

F32 = mybir.dt.float32
AF = mybir.ActivationFunctionType

# layer geometry: K input feats, F gate width, nf = F//128 full chunks,
# r = F%128 tail rows, stride = 32-aligned stack stride, s = stacks.
LAYERS = [
    dict(K=9, F=288, nf=2, r=32, stride=32, s=4),
    dict(K=288, F=576, nf=4, r=64, stride=64, s=2),
    dict(K=576, F=144, nf=1, r=16, stride=32, s=4),
]
MERGED_TAILS = bool(int(os.environ.get("KERNEL_MERGED_TAILS", "0")))
# merged-tail layout per layer: blocks of [u(r); o(r)] stacked s2-fold at
# stride2 partitions, covering cps = BT // s2 batch columns per stack.
TAILG = [
    dict(r=32, stride=64, s=2, oo=32),    # L1: [u32|o32]       -> [128, 1024]
    dict(r=64, stride=128, s=1, oo=64),   # L2: [u64|o64]       -> [128, 2048]
    dict(r=16, stride=64, s=2, oo=32),    # L3: [u16|pad16|o16] -> [128, 1024]
]
DENSE_K, DENSE_F = 144, 9
CH = [32, 64, 16]
CIN = [1, 32, 64]


# ---------------------------------------------------------------- host-side prep
def _conv_to_dense(w, cin_used):
    """w: [Cout, Cin, 3, 3] OIHW SAME conv on 3x3 images -> A: [Cout*9, cin_used*9]
    with y_flat = A @ x_flat, flat feature index = c*9 + i*3 + j."""
    w = np.asarray(w, np.float32)[:, :cin_used]
    cout, cin = w.shape[0], w.shape[1]
    A = np.zeros((cout, 9, cin, 9), np.float32)
    for i in range(3):
        for j in range(3):
            for di in range(3):
                for dj in range(3):
                    si, sj = i + di - 1, j + dj - 1
                    if 0 <= si < 3 and 0 <= sj < 3:
                        A[:, i * 3 + j, :, si * 3 + sj] = w[:, :, di, dj]
    return A.reshape(cout * 9, cin * 9)


def _bias_pack(b_chan, L):
    """Per-channel bias -> [128, nf+1] fp32: col m = features m*128..m*128+127,
    last col = tail features replicated at each stack's partition offset."""
    nf, r, stride, s = L["nf"], L["r"], L["stride"], L["s"]
    bf = np.repeat(np.asarray(b_chan, np.float32), 9)
    out = np.zeros((128, nf + 1), np.float32)
    for m in range(nf):
        out[:, m] = bf[m * 128:(m + 1) * 128]
    for k in range(s if STACK_TAILS else 1):
        out[k * stride:k * stride + r, nf] = bf[L["F"] - r:]
    return out


def _prep_consts(inputs):
    c = {}
    for li, L in enumerate(LAYERS, start=1):
        for g in ("u", "o"):
            A = _conv_to_dense(inputs[f"w{li}{g}"], CIN[li - 1])      # [F, K]
            c[f"w{li}{g}"] = np.ascontiguousarray(A.T).astype(MM_NP)  # [K, F]
            c[f"b{li}{g}"] = _bias_pack(inputs[f"b{li}{g}"], LAYERS[li - 1])
    c["wd"] = np.ascontiguousarray(
        np.asarray(inputs["wd"], np.float32).T).astype(MM_NP)          # [144, 9]
    bd = np.zeros((128, 1), np.float32)
    for k in range(4 if STACK_TAILS else 1):
        bd[k * 32:k * 32 + DENSE_F, 0] = np.asarray(inputs["bd"], np.float32)
    c["bd"] = bd
    if MERGED_TAILS:
        for li, (L, T) in enumerate(zip(LAYERS, TAILG), start=1):
            r, st2, s2 = T["r"], T["stride"], T["s"]
            bu = np.repeat(np.asarray(inputs[f"b{li}u"], np.float32), 9)[L["F"] - r:]
            bo = np.repeat(np.asarray(inputs[f"b{li}o"], np.float32), 9)[L["F"] - r:]
            oo = T["oo"]
            bm = np.zeros((128, 1), np.float32)
            for k in range(s2):
                bm[k * st2:k * st2 + r, 0] = bu
                bm[k * st2 + oo:k * st2 + oo + r, 0] = bo
            c[f"bm{li}"] = bm
            sel = np.zeros((128, r), MM_NP)
            for k in range(s2):
                for i in range(r):
                    sel[k * st2 + oo + i, i] = 1.0
            c[f"sel{li}"] = sel
    return c


def _full_chunks(K):
    """[(lo, hi), ...] covering the full-128 part of K."""
    return [(m * 128, (m + 1) * 128) for m in range(K // 128)]


# ---------------------------------------------------------------- bass kernel
def build_bass(b_core=B_CORE):
    nc = bacc.Bacc("TRN2", target_bir_lowering=False, debug=False)
    nbt = b_core // BT
    assert b_core % BT == 0

    # ---- DRAM tensors
    x0_d = nc.dram_tensor("x0", [9, b_core], MM_DT, kind="ExternalInput").ap()
    wd_d, bd_d = {}, {}
    for li, L in enumerate(LAYERS, start=1):
        for g in ("u", "o"):
            wd_d[f"{li}{g}"] = nc.dram_tensor(
                f"w{li}{g}", [L["K"], L["F"]], MM_DT, kind="ExternalInput").ap()
            bd_d[f"{li}{g}"] = nc.dram_tensor(
                f"b{li}{g}", [128, L["nf"] + 1], F32, kind="ExternalInput").ap()
    wdd = nc.dram_tensor("wd", [DENSE_K, DENSE_F], MM_DT, kind="ExternalInput").ap()
    bdd = nc.dram_tensor("bd", [128, 1], F32, kind="ExternalInput").ap()
    bm_d = {}
    if MERGED_TAILS:
        for li in (1, 2, 3):
            bm_d[li] = nc.dram_tensor(
                f"bm{li}", [128, 1], F32, kind="ExternalInput").ap()
            bm_d[(li, "sel")] = nc.dram_tensor(
                f"sel{li}", [128, TAILG[li - 1]["r"]], MM_DT,
                kind="ExternalInput").ap()
    x_out = [
        nc.dram_tensor(f"x{li}", [L["F"], b_core], F32, kind="ExternalOutput").ap()
        for li, L in enumerate(LAYERS, start=1)
    ]
    y_d = nc.dram_tensor("y", [DENSE_F, b_core], F32, kind="ExternalOutput").ap()

    # geometry of each matmul-input source, in order: L1 in, L2 in, L3 in, dense in
    tg = TAILG if MERGED_TAILS else [
        dict(r=L["r"], stride=L["stride"], s=L["s"]) for L in LAYERS]
    in_geo = [dict(full_rows=[9], r=0, stride=0, s=0)] + [
        dict(full_rows=[128] * L["nf"], r=T["r"], stride=T["stride"], s=T["s"])
        for L, T in zip(LAYERS, tg)
    ]

    with tile.TileContext(nc) as tc:
        with (
            tc.tile_pool(name="const", bufs=1) as constp,
            tc.tile_pool(name="psum", bufs=2, space="PSUM") as psump,
            tc.tile_pool(name="g16", bufs=int(os.environ.get("KERNEL_G16B", "12"))) as gatep,
            tc.tile_pool(name="h32", bufs=int(os.environ.get("KERNEL_H32B", "8"))) as h32p,
            tc.tile_pool(name="h16", bufs=int(os.environ.get("KERNEL_H16B", "16"))) as h16p,
            tc.tile_pool(name="outp", bufs=2) as outp,
        ):
            # ---- load constants.  Weight tiles per input K-chunk; the tail
            # K-chunk is loaded replicated at each stack's partition offset so
            # lhsT/rhs SBUF base partitions match (PE row-group requirement).
            x0_t = constp.tile([9, b_core], MM_DT)
            nc.sync.dma_start(x0_t[:], x0_d[:])

            def load_w(dram, K, F, geo, name):
                tiles = []
                for (lo, hi) in _full_chunks(K) if K >= 128 else [(0, K)]:
                    t = constp.tile([hi - lo, F], MM_DT, name=f"{name}_{lo}")
                    nc.sync.dma_start(t[:], dram[lo:hi, :])
                    tiles.append(t)
                tail = None
                if K >= 128 and K % 128:
                    r, stride, s = geo["r"], geo["stride"], geo["s"]
                    assert K % 128 == r
                    tail = constp.tile([128, F], MM_DT, name=f"{name}_tail")
                    for k in range(s if STACK_TAILS else 1):
                        nc.sync.dma_start(
                            tail[k * stride:k * stride + r, :], dram[K - r:K, :])
                return tiles, tail

            W, BIA = {}, {}
            for li, L in enumerate(LAYERS, start=1):
                for g in ("u", "o"):
                    W[(li, g)] = load_w(wd_d[f"{li}{g}"], L["K"], L["F"],
                                        in_geo[li - 1], f"w{li}{g}")
                    bt_ = constp.tile([128, L["nf"] + 1], F32, name=f"b{li}{g}")
                    nc.sync.dma_start(bt_[:], bd_d[f"{li}{g}"][:])
                    BIA[(li, g)] = bt_
            WM, BM = {}, {}
            if MERGED_TAILS:
                for li, (L, T) in enumerate(zip(LAYERS, TAILG), start=1):
                    r, oo = T["r"], T["oo"]
                    wcols = oo + r
                    tiles = []
                    kcs = (_full_chunks(L["K"]) if L["K"] >= 128
                           else [(0, L["K"])])
                    for (lo, hi) in kcs:
                        t = constp.tile([hi - lo, wcols], MM_DT,
                                        name=f"wm{li}_{lo}")
                        if oo != r:
                            nc.gpsimd.memset(t[:], 0.0)
                        nc.sync.dma_start(
                            t[:, 0:r], wd_d[f"{li}u"][lo:hi, L["F"] - r:])
                        nc.sync.dma_start(
                            t[:, oo:oo + r], wd_d[f"{li}o"][lo:hi, L["F"] - r:])
                        tiles.append(t)
                    tailw = None
                    if L["K"] >= 128 and L["K"] % 128:
                        pg = in_geo[li - 1]
                        tailw = constp.tile([128, wcols], MM_DT,
                                            name=f"wm{li}_tail")
                        if oo != r:
                            nc.gpsimd.memset(tailw[:], 0.0)
                        for k in range(pg["s"]):
                            o = k * pg["stride"]
                            nc.sync.dma_start(
                                tailw[o:o + pg["r"], 0:r],
                                wd_d[f"{li}u"][L["K"] - pg["r"]:, L["F"] - r:])
                            nc.sync.dma_start(
                                tailw[o:o + pg["r"], oo:oo + r],
                                wd_d[f"{li}o"][L["K"] - pg["r"]:, L["F"] - r:])
                    WM[li] = (tiles, tailw)
                    bmt = constp.tile([128, 1], F32, name=f"bm{li}")
                    nc.sync.dma_start(bmt[:], bm_d[li][:])
                    BM[li] = bmt
                    selt = constp.tile([128, T["r"]], MM_DT, name=f"sel{li}")
                    nc.sync.dma_start(selt[:], bm_d[(li, "sel")][:])
                    BM[(li, "sel")] = selt
            WDt = load_w(wdd, DENSE_K, DENSE_F, in_geo[3], "wd")
            BD = constp.tile([128, 1], F32, name="bd")
            nc.sync.dma_start(BD[:], bdd[:])

            # ---- emission helpers --------------------------------
            def mk_rhs_l1(c0):
                def _rhs(kc, j):
                    return x0_t[0:9, c0 + j * SUB:c0 + (j + 1) * SUB], 0
                return _rhs

            def mk_rhs(full, tail, nf_, r_, stride_, s_):
                cps_ = BT // s_ if STACK_TAILS else BT

                def _rhs(kc, j):
                    if kc < nf_:
                        return full[kc][:, j * SUB:(j + 1) * SUB], 0
                    if STACK_TAILS:
                        k = (j * SUB) // cps_
                        col = (j * SUB) % cps_
                        return (tail[k * stride_:k * stride_ + r_,
                                     col:col + SUB], k * stride_)
                    return tail[0:r_, j * SUB:(j + 1) * SUB], 0
                return _rhs

            def emit_layer(li, c0, rhs, geo, ms=None, state=None):
                """One gate layer of big-tile at batch column c0; ms selects
                a subset of chunk indices (default: all full chunks then the
                tail).  Returns (h16_full list, h16_tail), accumulated in
                `state` across partial calls."""
                L = LAYERS[li - 1]
                nf, r, stride, s = L["nf"], L["r"], L["stride"], L["s"]
                n_full = len(geo["full_rows"])
                if state is None:
                    state = {"full": [None] * nf, "tail": None}
                h16_full, h16_tail = state["full"], state["tail"]
                for m in (list(range(nf)) + [nf]) if ms is None else ms:
                    is_tail = m == nf
                    fcols = (slice(L["F"] - r, L["F"]) if is_tail
                             else slice(m * 128, (m + 1) * 128))
                    g16 = {}
                    for g, func in (("u", AF.Sigmoid), ("o", AF.Tanh)):
                        wf, wt = W[(li, g)]
                        if not is_tail:
                            ps = psump.tile([128, BT], F32, tag="ps")
                            regions = [(0, 128, 0, j, j * SUB)
                                       for j in range(BT // SUB)]
                        elif STACK_TAILS:
                            cps = BT // s
                            ps = psump.tile([128, cps], F32, tag="ps")
                            if r != stride and SIM_SAFE:
                                nc.vector.memset(ps[:], 0.0)
                            regions = []
                            for k in range(s):
                                for h in range(cps // SUB):
                                    j = (k * cps) // SUB + h
                                    regions.append((k * stride, r, k, j, h * SUB))
                        else:
                            ps = psump.tile([r, BT], F32, tag="ps")
                            regions = [(0, r, 0, j, j * SUB)
                                       for j in range(BT // SUB)]
                        # Full K-chunks first: the tail K-chunk is the
                        # *last* thing the previous layer produces.
                        kc_order = (list(range(n_full))
                                    + ([n_full] if geo["r"] else []))
                        # Loop order vs has_written safety:
                        #  - full chunks: every region writes the same
                        #    partitions to a *disjoint* bank, so kc-outer /
                        #    region-inner is safe under both the per-partition
                        #    sim model and whole-bank bit clears -- and it
                        #    reuses the stationary weights across the 4
                        #    regions (4x fewer LDWEIGHTS on hardware; the
                        #    cost model does not charge LDWEIGHTS at all).
                        #  - stacked tails: regions share banks at different
                        #    partition offsets; each region's accumulation
                        #    group must complete before the next region
                        #    starts, so keep region-outer / kc-inner there.
                        if not is_tail or not STACK_TAILS:
                            for kc in kc_order:
                                rr0 = None
                                for (plo, psz, kstk, j, coff) in regions:
                                    rr, rbase = rhs(kc, j)
                                    if kc < n_full:
                                        lhs = wf[kc][0:geo["full_rows"][kc], fcols]
                                    else:
                                        lhs = wt[rbase:rbase + geo["r"], fcols]
                                    nc.tensor.matmul(
                                        ps[plo:plo + psz, coff:coff + SUB],
                                        lhs, rr,
                                        start=(kc == kc_order[0]),
                                        stop=(kc == kc_order[-1]),
                                        tile_position=(rbase, plo),
                                        skip_group_check=True)
                        else:
                            for (plo, psz, kstk, j, coff) in regions:
                                for kc in kc_order:
                                    rr, rbase = rhs(kc, j)
                                    if kc < n_full:
                                        lhs = wf[kc][0:geo["full_rows"][kc], fcols]
                                    else:
                                        lhs = wt[rbase:rbase + geo["r"], fcols]
                                    nc.tensor.matmul(
                                        ps[plo:plo + psz, coff:coff + SUB],
                                        lhs, rr,
                                        start=(kc == kc_order[0]),
                                        stop=(kc == kc_order[-1]),
                                        tile_position=(rbase, plo),
                                        skip_group_check=True)
                        gt = gatep.tile(list(ps.shape), MM_DT, tag="g16")
                        bcol = slice(nf, nf + 1) if is_tail else slice(m, m + 1)
                        if is_tail and STACK_TAILS and SPLIT_TAILS:
                            for k in range(s):
                                sl = slice(k * stride, k * stride + r)
                                nc.scalar.activation(
                                    gt[sl, :], ps[sl, :], func,
                                    bias=BIA[(li, g)][sl, bcol])
                        else:
                            bias = BIA[(li, g)][0:ps.shape[0], bcol]
                            nc.scalar.activation(gt[:], ps[:], func, bias=bias)
                        g16[g] = gt
                    shape = list(g16["u"].shape)
                    h32 = h32p.tile(shape, F32, tag="h32")
                    h16 = h16p.tile(shape, MM_DT, tag="h16")
                    # h16 (the next layer's input, the latency-critical one)
                    # is produced FIRST as a direct fp16-out multiply; the
                    # fp32 product for the DRAM store follows off the critical
                    # path.  Both are the same DVE fp32-internal product, so
                    # h16 == cast(h32) exactly.
                    # Full-width even when r != stride: the unwritten gap
                    # rows carry junk that nothing reads (stores and the next
                    # layer's rhs slice valid rows only); one 128-lane
                    # instruction replaces s narrow ones.
                    if is_tail and STACK_TAILS and SPLIT_TAILS:
                        for k in range(s):
                            sl = slice(k * stride, k * stride + r)
                            nc.vector.tensor_mul(h16[sl, :], g16["o"][sl, :],
                                                 g16["u"][sl, :])
                        for k in range(s):
                            sl = slice(k * stride, k * stride + r)
                            nc.vector.tensor_mul(h32[sl, :], g16["o"][sl, :],
                                                 g16["u"][sl, :])
                    else:
                        nc.vector.tensor_mul(h16[:], g16["o"][:], g16["u"][:])
                        nc.vector.tensor_mul(h32[:], g16["o"][:], g16["u"][:])
                    if not is_tail:
                        nc.gpsimd.dma_start(
                            x_out[li - 1][m * 128:(m + 1) * 128, c0:c0 + BT],
                            h32[:])
                        h16_full[m] = h16
                    elif STACK_TAILS:
                        cps = BT // s
                        for k in range(s):
                            nc.gpsimd.dma_start(
                                x_out[li - 1][L["F"] - r:L["F"],
                                              c0 + k * cps:c0 + (k + 1) * cps],
                                h32[k * stride:k * stride + r, :])
                        h16_tail = h16
                        state["tail"] = h16
                    else:
                        nc.gpsimd.dma_start(
                            x_out[li - 1][L["F"] - r:L["F"], c0:c0 + BT],
                            h32[:])
                        h16_tail = h16
                state["full"], state["tail"] = h16_full, h16_tail
                return h16_full, h16_tail

            def emit_merged_tail(li, c0, rhs, geo, state):
                """Both gates' tail features (r each) in ONE M-chunk of 2r
                rows, batch-stacked s2-fold: halves the tail matmul passes.
                Layout per stack k: rows [k*st2, k*st2+r) = u-part,
                [k*st2+r, k*st2+2r) = o-part.  The product needs u and o at
                the same partition base, which DVE requires, so the o-part is
                realigned with a small SBUF->SBUF DMA (Pool engine) first."""
                L, T = LAYERS[li - 1], TAILG[li - 1]
                r, st2, s2, oo = T["r"], T["stride"], T["s"], T["oo"]
                cps = BT // s2
                n_full = len(geo["full_rows"])
                wf, wt = WM[li]
                ps = psump.tile([128, cps], F32, tag="ps")
                regions = []
                for k in range(s2):
                    for h in range(cps // SUB):
                        j = (k * cps) // SUB + h
                        regions.append((k * st2, j, h * SUB))
                kc_order = list(range(n_full)) + ([n_full] if geo["r"] else [])

                def mm(plo, j, coff, kc):
                    rr, rbase = rhs(kc, j)
                    if kc < n_full:
                        lhs = wf[kc][0:geo["full_rows"][kc], 0:oo + r]
                    else:
                        lhs = wt[rbase:rbase + geo["r"], 0:oo + r]
                    nc.tensor.matmul(
                        ps[plo:plo + oo + r, coff:coff + SUB], lhs, rr,
                        start=(kc == kc_order[0]), stop=(kc == kc_order[-1]),
                        tile_position=(rbase, plo), skip_group_check=True)

                if s2 == 1:
                    # regions are disjoint banks on the same partitions:
                    # kc-outer is safe and reuses the stationary weights.
                    for kc in kc_order:
                        for (plo, j, coff) in regions:
                            mm(plo, j, coff, kc)
                else:
                    # stacks share banks at different partitions: complete
                    # each region's accumulation group before the next.
                    for (plo, j, coff) in regions:
                        for kc in kc_order:
                            mm(plo, j, coff, kc)

                gt = gatep.tile([128, cps], MM_DT, tag="g16")
                for k in range(s2):
                    o = k * st2
                    nc.scalar.activation(gt[o:o + r, :], ps[o:o + r, :],
                                         AF.Sigmoid, bias=BM[li][o:o + r, 0:1])
                    nc.scalar.activation(gt[o + oo:o + oo + r, :],
                                         ps[o + oo:o + oo + r, :],
                                         AF.Tanh,
                                         bias=BM[li][o + oo:o + oo + r, 0:1])
                selt = BM[(li, "sel")]

                def finish():
                    # PE realign: one-hot matmul moves each stack's tanh'd
                    # o-part into the (dead, already-ACT'd) u-rows of the
                    # tail psum tile, so the product is base-aligned for DVE.
                    for k in range(s2):
                        o = k * st2
                        for h in range(cps // SUB):
                            nc.tensor.matmul(
                                ps[o:o + r, h * SUB:(h + 1) * SUB],
                                selt[o + oo:o + oo + r, 0:r],
                                gt[o + oo:o + oo + r, h * SUB:(h + 1) * SUB],
                                start=True, stop=True,
                                tile_position=(o + oo, o),
                                skip_group_check=True)
                    h16 = h16p.tile([128, cps], MM_DT, tag="h16")
                    h32 = h32p.tile([128, cps], F32, tag="h32")
                    for k in range(s2):
                        o = k * st2
                        nc.vector.tensor_mul(h16[o:o + r, :], gt[o:o + r, :],
                                             ps[o:o + r, :])
                    for k in range(s2):
                        o = k * st2
                        nc.vector.tensor_mul(h32[o:o + r, :], gt[o:o + r, :],
                                             ps[o:o + r, :])
                    for k in range(s2):
                        nc.gpsimd.dma_start(
                            x_out[li - 1][L["F"] - r:L["F"],
                                          c0 + k * cps:c0 + (k + 1) * cps],
                            h32[k * st2:k * st2 + r, :])
                    state["tail"] = h16
                return finish

            def emit_gate_layer(li, c0, rhs, geo):
                nf = LAYERS[li - 1]["nf"]
                if MERGED_TAILS:
                    # tail MMs+ACTs first; the realign+product lands after
                    # the first full chunk so the ACT(o) -> realign dep never
                    # head-of-line-blocks the PE queue.
                    st = {"full": [None] * nf, "tail": None}
                    fin = emit_merged_tail(li, c0, rhs, geo, st)
                    emit_layer(li, c0, rhs, geo, ms=[0], state=st)
                    fin()
                    if nf > 1:
                        emit_layer(li, c0, rhs, geo,
                                   ms=list(range(1, nf)), state=st)
                    return st["full"], st["tail"]
                # tail group second (after full chunk 0): its ACT->mul chain
                # completes while the remaining full chunks run, so the next
                # layer's tail K-chunk is ready when consumers reach it.
                return emit_layer(li, c0, rhs, geo,
                                  ms=[0, nf] + list(range(1, nf)))

            def emit_dense(c0, rhs, geo):
                n_full = len(geo["full_rows"])
                wf, wt = WDt
                if STACK_TAILS:
                    ps = psump.tile([128, SUB], F32, tag="ps")
                    if SIM_SAFE:
                        nc.vector.memset(ps[:], 0.0)
                    dj = [(j * 32, j, 0) for j in range(BT // SUB)]
                else:
                    ps = psump.tile([DENSE_F, BT], F32, tag="ps")
                    dj = [(0, j, j * SUB) for j in range(BT // SUB)]
                kc_order = list(range(n_full)) + [n_full]
                for (plo, j, coff) in dj:
                    for kc in kc_order:
                        rr, rbase = rhs(kc, j)
                        if kc < n_full:
                            lhs = wf[kc][:, 0:DENSE_F]
                        else:
                            lhs = wt[rbase:rbase + geo["r"], 0:DENSE_F]
                        nc.tensor.matmul(
                            ps[plo:plo + DENSE_F, coff:coff + SUB], lhs, rr,
                            start=(kc == kc_order[0]), stop=(kc == kc_order[-1]),
                            tile_position=(rbase, plo),
                            skip_group_check=True)
                o32 = outp.tile(list(ps.shape), F32, tag="o32")
                if STACK_TAILS:
                    nc.vector.tensor_scalar_add(o32[:], ps[:], BD[:, 0:1])
                    for (plo, j, coff) in dj:
                        nc.sync.dma_start(
                            y_d[0:DENSE_F, c0 + j * SUB:c0 + (j + 1) * SUB],
                            o32[plo:plo + DENSE_F, :])
                else:
                    nc.vector.tensor_scalar_add(o32[:], ps[:], BD[0:DENSE_F, 0:1])
                    nc.sync.dma_start(y_d[0:DENSE_F, c0:c0 + BT], o32[:])

            # ---- skewed software pipeline: L1 runs one big-tile ahead, so
            # its (independent-of-everything) matmuls fill the PE stalls at
            # the dense layer barriers of the previous big-tile.
            l1_out = {}
            SCHED = os.environ.get("KERNEL_SCHED", "A")
            NF1 = LAYERS[0]["nf"]

            def l1_pieces(w):
                c0 = w * BT
                rhs1 = mk_rhs_l1(c0)
                st = {"full": [None] * NF1, "tail": None}
                l1_out[w] = st
                ps = [
                    (lambda m=m: emit_layer(1, c0, rhs1, in_geo[0],
                                            ms=[m], state=st))
                    for m in range(NF1)
                ]
                if MERGED_TAILS:
                    fin_box = {}

                    def tail_mm():
                        fin_box["f"] = emit_merged_tail(1, c0, rhs1,
                                                        in_geo[0], st)

                    ps.insert(0, tail_mm)
                    ps.insert(2, lambda: fin_box["f"]())
                else:
                    ps.insert(1, lambda: emit_layer(1, c0, rhs1, in_geo[0],
                                                    ms=[NF1], state=st))
                return ps

            def emit_rest(b, pieces=()):
                # The next big-tile's L1 chunk-groups are emitted right after
                # each layer of this big-tile: their priority then sits
                # exactly at the layer-barrier stalls, giving PE independent
                # fill work while the barrier's ACT->mul->cast chain drains.
                c0 = b * BT
                L1, L2, L3 = LAYERS
                T1, T2, T3 = tg
                st = l1_out.pop(b)
                rhs = mk_rhs(st["full"], st["tail"],
                             L1["nf"], T1["r"], T1["stride"], T1["s"])
                hf, ht = emit_gate_layer(2, c0, rhs, in_geo[1])
                if len(pieces) > 0:
                    pieces[0]()
                rhs = mk_rhs(hf, ht, L2["nf"], T2["r"], T2["stride"], T2["s"])
                hf, ht = emit_gate_layer(3, c0, rhs, in_geo[2])
                if len(pieces) > 1:
                    pieces[1]()
                rhs = mk_rhs(hf, ht, L3["nf"], T3["r"], T3["stride"], T3["s"])
                emit_dense(c0, rhs, in_geo[3])
                for p in pieces[2:]:
                    p()

            if SCHED == "I":
                for w in range(nbt + 1):
                    pieces = l1_pieces(w) if w < nbt else ()
                    if w == 0:
                        for p in pieces:
                            p()
                    else:
                        emit_rest(w - 1, pieces)
            elif SCHED == "A2":
                # L1 runs TWO big-tiles ahead: twice the independent fill
                # inventory at the dense layer barriers.
                for ww in range(min(2, nbt)):
                    for p in l1_pieces(ww):
                        p()
                for b in range(nbt):
                    if b + 2 < nbt:
                        for p in l1_pieces(b + 2):
                            p()
                    emit_rest(b)
            else:
                for w in range(nbt + 1):
                    if w < nbt:
                        pieces = l1_pieces(w)
                        for p in pieces:
                            p()
                    if w >= 1:
                        emit_rest(w - 1)

    nc.compile()
    return nc


_NC_CACHE = {}


def _get_nc(b_core):
    if b_core not in _NC_CACHE:
        _NC_CACHE[b_core] = build_bass(b_core)
    return _NC_CACHE[b_core]


# ---------------------------------------------------------------- entry point
def kernel(**inputs):
    consts = _prep_consts(inputs)
    x = np.asarray(inputs["inputs"], np.float32).reshape(B_TOTAL, 9)

    in_maps = []
    for c in range(N_CORES):
        m = dict(consts)
        xc = x[c * B_CORE:(c + 1) * B_CORE]
        m["x0"] = np.ascontiguousarray(xc.T).astype(MM_NP)
        in_maps.append(m)

    nc = _get_nc(B_CORE)
    trace = bool(int(os.environ.get("KERNEL_TRACE", "0")))
    res = run_bass_kernel_spmd(nc, in_maps, core_ids=list(range(N_CORES)),
                               trace=trace)
    if trace and res.exec_time_ns is not None:
        print(f"HW exec time: {res.exec_time_ns} ns")
        kernel.last_exec_time_ns = res.exec_time_ns

    outs, x1s, x2s, x3s = [], [], [], []
    for c in range(N_CORES):
        r = res.results[c]
        outs.append(np.ascontiguousarray(r["y"].T))
        x1s.append(np.ascontiguousarray(r["x1"].T).reshape(B_CORE, 32, 3, 3))
        x2s.append(np.ascontiguousarray(r["x2"].T).reshape(B_CORE, 64, 3, 3))
        x3s.append(np.ascontiguousarray(r["x3"].T).reshape(B_CORE, 16, 3, 3))
    return (np.concatenate(outs), np.concatenate(x1s),
            np.concatenate(x2s), np.concatenate(x3s))
